# revision 6
# baseline (speedup 1.0000x reference)
"""Trainium2 Bass kernel for an 8-expert top-2 MoE layer.

Problem (hardcoded): x[8,2048,1024] f32, gate Wg[1024,8]+bg, experts
W1[8,1024,2048]+b1, W2[8,2048,1024]+b2, top-2 routing with renormalized
gate weights, out[8,2048,1024] f32.

Strategy: data-parallel over tokens. Each of the 8 NeuronCores processes one
batch row (2048 tokens) with all experts resident:
  1. gate logits via PE (fp32), top-2 + weights via DVE max8,
  2. build per-expert token lists on-device (one-hot transpose -> free-axis
     cumsum -> positions -> indirect scatter of token ids),
  3. per expert: dma_gather(transpose) dispatches routed tokens into a
     [D,tok] bf16 activation panel; two bf16 matmuls (weights stationary as
     lhsT) with fused bias+ReLU eviction; xbar DMA-transpose back to
     token-major; linear store into a [expert-slot, D] bf16 workspace,
  4. final combine: per token dma_gather of its two expert rows, scale by
     gate weights in fp32, store.
The capacity per (core, expert) is CAP=640 slots (mean load 512); overflow
beyond CAP is clamped into an unprocessed spill slot (probability ~0 for
gaussian inputs).
"""

import sys

for _p in ("/opt/trn_rl_repo",):
    if _p not in sys.path:
        sys.path.append(_p)

import numpy as np
import ml_dtypes

import concourse.bass as bass
import concourse.bacc as bacc
import concourse.tile as tile
import concourse.mybir as mybir
from concourse.masks import make_identity

P = 128
B, S, D = 8, 2048, 1024
E, H, TOPK = 8, 2048, 2
T = S  # tokens per core (one batch row per core)
NT = T // P  # 16 token tiles
KD = D // P  # 8 contraction tiles for D
KH = H // P  # 16 contraction tiles for H
MH = H // P  # 16 output tiles for H
MD = D // P  # 8 output tiles for D
CAP = 640  # processed slots per (core, expert)
STRIDE = 656  # idxlist/ybuf row stride per expert (CAP + spill)
XROWS = T + 16  # xb pad rows; row T is the all-zero dump row
YROWS = E * STRIDE
NCH = ((0, 512), (512, 128))  # token chunks of CAP for PSUM banks
DT = mybir.dt


def build_program():
    nc = bacc.Bacc("TRN2", target_bir_lowering=False, debug=False, num_devices=8)

    xt = nc.dram_tensor("xt", [D, T], DT.float32, kind="ExternalInput").ap()
    xb = nc.dram_tensor("xb", [XROWS, D], DT.bfloat16, kind="ExternalInput").ap()
    wg = nc.dram_tensor("wg", [P, KD * E], DT.float32, kind="ExternalInput").ap()
    bgb = nc.dram_tensor("bgb", [P, E], DT.float32, kind="ExternalInput").ap()
    iotae = nc.dram_tensor("iotae", [P, E], DT.float32, kind="ExternalInput").ap()
    tokid = nc.dram_tensor("tokid", [P, NT], DT.int16, kind="ExternalInput").ap()
    w1l = nc.dram_tensor("w1l", [E, P, KD * H], DT.bfloat16, kind="ExternalInput").ap()
    w2l = nc.dram_tensor("w2l", [E, P, KH * D], DT.bfloat16, kind="ExternalInput").ap()
    b1l = nc.dram_tensor("b1l", [E, P, MH], DT.float32, kind="ExternalInput").ap()
    b2l = nc.dram_tensor("b2l", [E, P, MD], DT.float32, kind="ExternalInput").ap()
    out = nc.dram_tensor("out", [T, D], DT.float32, kind="ExternalOutput").ap()

    idxlist = nc.dram_tensor("idxlist", [YROWS, 1], DT.int16).ap()
    gbuf = nc.dram_tensor("gbuf", [2, T], DT.int16).ap()
    ybuf = nc.dram_tensor("ybuf", [YROWS, D], DT.bfloat16).ap()

    with tile.TileContext(nc) as tc, tc.tile_pool(name="pers", bufs=1) as pers:
        with (
            tc.tile_pool(name="route", bufs=3) as route,
            tc.tile_pool(name="routeps", bufs=2, space="PSUM") as routeps,
            tc.tile_pool(name="gateps", bufs=2, space="PSUM") as gateps,
            tc.tile_pool(name="xtp", bufs=3) as xtp,
        ):
            ident = pers.tile([P, P], DT.float32)
            make_identity(nc, ident)
            wg_sb = pers.tile([P, KD * E], DT.float32)
            nc.sync.dma_start(wg_sb[:], wg[:, :])
            bgb_sb = pers.tile([P, E], DT.float32)
            nc.sync.dma_start(bgb_sb[:], bgb[:, :])
            iot_sb = pers.tile([P, E], DT.float32)
            nc.sync.dma_start(iot_sb[:], iotae[:, :])
            tok_sb = pers.tile([P, NT], DT.int16)
            nc.sync.dma_start(tok_sb[:], tokid[:, :])

            # init idxlist to the dump token id (T -> zero row of xb)
            init_t = pers.tile([P, YROWS // P], DT.int16)
            nc.vector.memset(init_t[:], T)
            nc.sync.dma_start(idxlist.rearrange("(p c) o -> p (c o)", p=P), init_t[:])

            # zero the spill rows of ybuf (rows e*STRIDE+CAP .. e*STRIDE+655)
            zspill = pers.tile([16, D], DT.bfloat16)
            nc.vector.memset(zspill[:], 0)
            for e in range(E):
                nc.sync.dma_start(
                    ybuf[e * STRIDE + CAP : e * STRIDE + STRIDE, :], zspill[:]
                )

            combT = pers.tile([E, T], DT.float32)
            zerosE = pers.tile([E, T], DT.float32)
            nc.vector.memset(zerosE[:], 0.0)
            i1a = pers.tile([P, NT], DT.float32)
            i2a = pers.tile([P, NT], DT.float32)
            w1a = pers.tile([P, NT], DT.float32)
            w2a = pers.tile([P, NT], DT.float32)

            # ---- gate + top-2 per token tile ----
            for i in range(NT):
                xt_sb = xtp.tile([P, KD, P], DT.float32)
                nc.sync.dma_start(
                    xt_sb[:],
                    xt.rearrange("(k p) t -> p k t", p=P)[
                        :, :, i * P : (i + 1) * P
                    ],
                )
                ps_g = gateps.tile([P, E], DT.float32)
                for k in range(KD):
                    nc.tensor.matmul(
                        ps_g[:],
                        lhsT=xt_sb[:, k, :],
                        rhs=wg_sb[:, k * E : (k + 1) * E],
                        start=(k == 0),
                        stop=(k == KD - 1),
                    )
                logits = route.tile([P, E], DT.float32)
                nc.vector.tensor_add(logits[:], ps_g[:], bgb_sb[:])

                vals8 = route.tile([P, 8], DT.float32)
                idx8 = route.tile([P, 8], DT.uint32)
                nc.vector.max_with_indices(vals8[:], idx8[:], logits[:])
                nc.vector.tensor_copy(i1a[:, i : i + 1], idx8[:, 0:1])
                nc.vector.tensor_copy(i2a[:, i : i + 1], idx8[:, 1:2])

                oh1 = route.tile([P, E], DT.float32)
                nc.vector.tensor_scalar(
                    oh1[:], iot_sb[:], i1a[:, i : i + 1], None,
                    op0=mybir.AluOpType.is_equal,
                )
                oh2 = route.tile([P, E], DT.float32)
                nc.vector.tensor_scalar(
                    oh2[:], iot_sb[:], i2a[:, i : i + 1], None,
                    op0=mybir.AluOpType.is_equal,
                )
                comb = route.tile([P, E], DT.float32)
                nc.vector.tensor_add(comb[:], oh1[:], oh2[:])
                ps_t = routeps.tile([E, P], DT.float32)
                nc.tensor.transpose(ps_t[:], comb[:, :], ident[:])
                nc.vector.tensor_copy(combT[:, i * P : (i + 1) * P], ps_t[:])

                dm = route.tile([P, 1], DT.float32)
                nc.vector.tensor_sub(dm[:], vals8[:, 1:2], vals8[:, 0:1])
                ed = route.tile([P, 1], DT.float32)
                nc.scalar.activation(ed[:], dm[:], mybir.ActivationFunctionType.Exp)
                den = route.tile([P, 1], DT.float32)
                nc.vector.tensor_scalar_add(den[:], ed[:], 1.0)
                nc.vector.reciprocal(w1a[:, i : i + 1], den[:])
                nc.vector.tensor_mul(w2a[:, i : i + 1], ed[:], w1a[:, i : i + 1])

            # ---- cumulative per-expert counts -> slot positions ----
            incl = pers.tile([E, T], DT.float32)
            nc.vector.tensor_tensor_scan(
                incl[:], combT[:], zerosE[:], 0.0,
                op0=mybir.AluOpType.add, op1=mybir.AluOpType.add,
            )
            excl = pers.tile([E, T], DT.float32)
            nc.vector.tensor_sub(excl[:], incl[:], combT[:])

            for i in range(NT):
                ps_e = routeps.tile([P, E], DT.float32)
                nc.tensor.transpose(
                    ps_e[:], excl[:, i * P : (i + 1) * P], ident[0:E, 0:E]
                )
                excl_tok = route.tile([P, E], DT.float32)
                nc.vector.tensor_copy(excl_tok[:], ps_e[:])
                for slot, ifc in ((0, i1a), (1, i2a)):
                    oh = route.tile([P, E], DT.float32)
                    nc.vector.tensor_scalar(
                        oh[:], iot_sb[:], ifc[:, i : i + 1], None,
                        op0=mybir.AluOpType.is_equal,
                    )
                    tmp = route.tile([P, E], DT.float32)
                    nc.vector.tensor_mul(tmp[:], excl_tok[:], oh[:])
                    ppos = route.tile([P, 1], DT.float32)
                    nc.vector.tensor_reduce(
                        ppos[:], tmp[:], axis=mybir.AxisListType.X,
                        op=mybir.AluOpType.add,
                    )
                    pm = route.tile([P, 1], DT.float32)
                    nc.vector.tensor_scalar_min(pm[:], ppos[:], float(CAP))
                    g = route.tile([P, 1], DT.float32)
                    nc.vector.tensor_scalar(
                        g[:], ifc[:, i : i + 1], float(STRIDE), pm[:, 0:1],
                        op0=mybir.AluOpType.mult, op1=mybir.AluOpType.add,
                    )
                    gi = route.tile([P, 1], DT.int32)
                    nc.vector.tensor_copy(gi[:], g[:])
                    nc.gpsimd.indirect_dma_start(
                        out=idxlist,
                        out_offset=bass.IndirectOffsetOnAxis(ap=gi[:, 0:1], axis=0),
                        in_=tok_sb[:, i : i + 1],
                        in_offset=None,
                    )
                    gs = route.tile([P, 1], DT.int16)
                    nc.vector.tensor_copy(gs[:], g[:])
                    nc.sync.dma_start(
                        gbuf[slot : slot + 1, i * P : (i + 1) * P], gs[:, 0:1]
                    )

        # ---- expert loop ----
        with (
            tc.tile_pool(name="w1p", bufs=10) as w1p,
            tc.tile_pool(name="w2p", bufs=18) as w2p,
            tc.tile_pool(name="bp", bufs=2) as bp,
            tc.tile_pool(name="idxp", bufs=2) as idxp,
            tc.tile_pool(name="xg", bufs=2) as xg,
            tc.tile_pool(name="hp", bufs=1) as hp,
            tc.tile_pool(name="yp", bufs=2) as yp,
            tc.tile_pool(name="ytk", bufs=3) as ytk,
            tc.tile_pool(name="mm1ps", bufs=2, space="PSUM") as mm1ps,
            tc.tile_pool(name="mm2ps", bufs=2, space="PSUM") as mm2ps,
        ):
            for e in range(E):
                idx_sb = idxp.tile([P, CAP // 16], DT.int16)
                with nc.allow_non_contiguous_dma(reason="tiny wrapped idx load"):
                    # idx block [16, n/16] must be replicated across the 8
                    # gpsimd core groups (HW reads its own 16-partition group)
                    for grp in range(8):
                        nc.sync.dma_start(
                            idx_sb[grp * 16 : grp * 16 + 16, :],
                            idxlist[e * STRIDE : e * STRIDE + CAP, 0].rearrange(
                                "(c p) -> p c", p=16
                            ),
                        )
                xgT = xg.tile([P, KD, CAP], DT.bfloat16)
                nc.gpsimd.dma_gather(
                    out_ap=xgT[:],
                    in_ap=xb[:, :],
                    idxs_ap=idx_sb[:, :],
                    num_idxs=CAP,
                    num_idxs_reg=CAP,
                    elem_size=D,
                    transpose=True,
                )

                w1k = []
                for k in range(KD):
                    wt = w1p.tile([P, H], DT.bfloat16, tag="w1")
                    nc.sync.dma_start(wt[:], w1l[e, :, k * H : (k + 1) * H])
                    w1k.append(wt)
                w2k = []
                for k in range(KH):
                    wt = w2p.tile([P, D], DT.bfloat16, tag="w2")
                    nc.sync.dma_start(wt[:], w2l[e, :, k * D : (k + 1) * D])
                    w2k.append(wt)
                b1t = bp.tile([P, MH], DT.float32, tag="b1")
                nc.sync.dma_start(b1t[:], b1l[e, :, :])
                b2t = bp.tile([P, MD], DT.float32, tag="b2")
                nc.sync.dma_start(b2t[:], b2l[e, :, :])

                hT = [
                    hp.tile([P, CAP], DT.bfloat16, tag=f"hT{m}", name=f"hT{m}")
                    for m in range(MH)
                ]
                for m in range(MH):
                    for n0, nsz in NCH:
                        ps1 = mm1ps.tile([P, 512], DT.float32)
                        for k in range(KD):
                            nc.tensor.matmul(
                                ps1[:, :nsz],
                                lhsT=w1k[k][:, m * P : (m + 1) * P],
                                rhs=xgT[:, k, n0 : n0 + nsz],
                                start=(k == 0),
                                stop=(k == KD - 1),
                            )
                        nc.scalar.activation(
                            hT[m][:, n0 : n0 + nsz],
                            ps1[:, :nsz],
                            mybir.ActivationFunctionType.Relu,
                            bias=b1t[:, m : m + 1],
                        )

                yT = [
                    yp.tile([P, CAP], DT.bfloat16, tag=f"yT{md}", name=f"yT{md}")
                    for md in range(MD)
                ]
                for md in range(MD):
                    for n0, nsz in NCH:
                        ps2 = mm2ps.tile([P, 512], DT.float32)
                        for k in range(KH):
                            nc.tensor.matmul(
                                ps2[:, :nsz],
                                lhsT=w2k[k][:, md * P : (md + 1) * P],
                                rhs=hT[k][:, n0 : n0 + nsz],
                                start=(k == 0),
                                stop=(k == KH - 1),
                            )
                        nc.scalar.activation(
                            yT[md][:, n0 : n0 + nsz],
                            ps2[:, :nsz],
                            mybir.ActivationFunctionType.Identity,
                            bias=b2t[:, md : md + 1],
                        )

                for j in range(CAP // P):
                    ytok = ytk.tile([P, D], DT.bfloat16)
                    for md in range(MD):
                        nc.sync.dma_start_transpose(
                            ytok[:, md * P : (md + 1) * P],
                            yT[md][:, j * P : (j + 1) * P],
                        )
                    nc.sync.dma_start(
                        ybuf[e * STRIDE + j * P : e * STRIDE + (j + 1) * P, :],
                        ytok[:],
                    )

        # ---- final combine ----
        with (
            tc.tile_pool(name="fin", bufs=4) as fin,
            tc.tile_pool(name="fing", bufs=4) as fing,
        ):
            for i in range(NT):
                gth = []
                for slot in range(2):
                    gidx = fin.tile([P, P // 16], DT.int16, tag="gidx")
                    with nc.allow_non_contiguous_dma(reason="tiny wrapped idx load"):
                        for grp in range(8):
                            nc.sync.dma_start(
                                gidx[grp * 16 : grp * 16 + 16, :],
                                gbuf[slot, i * P : (i + 1) * P].rearrange(
                                    "(c p) -> p c", p=16
                                ),
                            )
                    gt = fing.tile([P, 1, D], DT.bfloat16, tag=f"gth{slot}")
                    nc.gpsimd.dma_gather(
                        out_ap=gt[:],
                        in_ap=ybuf[:, :],
                        idxs_ap=gidx[:, :],
                        num_idxs=P,
                        num_idxs_reg=P,
                        elem_size=D,
                    )
                    gth.append(gt)
                acc1 = fin.tile([P, D], DT.float32, tag="acc1")
                nc.vector.tensor_scalar(
                    acc1[:], gth[0][:, 0, :], w1a[:, i : i + 1], None,
                    op0=mybir.AluOpType.mult,
                )
                acc2 = fin.tile([P, D], DT.float32, tag="acc2")
                nc.vector.tensor_scalar(
                    acc2[:], gth[1][:, 0, :], w2a[:, i : i + 1], None,
                    op0=mybir.AluOpType.mult,
                )
                res = fin.tile([P, D], DT.float32, tag="res")
                nc.vector.tensor_add(res[:], acc1[:], acc2[:])
                nc.sync.dma_start(out[i * P : (i + 1) * P, :], res[:])

    nc.compile()
    return nc


def prep_inputs(x, Wg, bg, W1, b1, W2, b2):
    """Build the 8 per-core input maps from full problem inputs (numpy f32)."""
    bf16 = ml_dtypes.bfloat16
    wg_l = np.ascontiguousarray(
        Wg.reshape(KD, P, E).transpose(1, 0, 2).reshape(P, KD * E)
    )
    bgb_np = np.tile(bg[None, :], (P, 1)).astype(np.float32)
    iot_np = np.tile(np.arange(E, dtype=np.float32)[None, :], (P, 1))
    tok_np = np.arange(T, dtype=np.int16).reshape(NT, P).T.copy()
    w1l_np = np.ascontiguousarray(
        W1.reshape(E, KD, P, H).transpose(0, 2, 1, 3).reshape(E, P, KD * H)
    ).astype(bf16)
    w2l_np = np.ascontiguousarray(
        W2.reshape(E, KH, P, D).transpose(0, 2, 1, 3).reshape(E, P, KH * D)
    ).astype(bf16)
    b1l_np = np.ascontiguousarray(
        b1.reshape(E, MH, P).transpose(0, 2, 1)
    ).astype(np.float32)
    b2l_np = np.ascontiguousarray(
        b2.reshape(E, MD, P).transpose(0, 2, 1)
    ).astype(np.float32)

    in_maps = []
    for c in range(B):
        xc = np.asarray(x[c], dtype=np.float32)  # [T, D]
        xt_np = np.ascontiguousarray(xc.T)
        xb_np = np.zeros((XROWS, D), dtype=bf16)
        xb_np[:T] = xc.astype(bf16)
        in_maps.append(
            {
                "xt": xt_np,
                "xb": xb_np,
                "wg": wg_l,
                "bgb": bgb_np,
                "iotae": iot_np,
                "tokid": tok_np,
                "w1l": w1l_np,
                "w2l": w2l_np,
                "b1l": b1l_np,
                "b2l": b2l_np,
            }
        )
    return in_maps


_nc_cache = None


def kernel(**inputs):
    global _nc_cache
    from concourse.bass_utils import run_bass_kernel_spmd

    if _nc_cache is None:
        _nc_cache = build_program()
    nc = _nc_cache
    in_maps = prep_inputs(
        np.asarray(inputs["x"], dtype=np.float32),
        np.asarray(inputs["Wg"], dtype=np.float32),
        np.asarray(inputs["bg"], dtype=np.float32),
        np.asarray(inputs["W1"], dtype=np.float32),
        np.asarray(inputs["b1"], dtype=np.float32),
        np.asarray(inputs["W2"], dtype=np.float32),
        np.asarray(inputs["b2"], dtype=np.float32),
    )
    res = run_bass_kernel_spmd(nc, in_maps, core_ids=list(range(B)))
    out = np.stack([res.results[c]["out"] for c in range(B)], axis=0)
    return out.astype(np.float32)


# revision 9
# speedup vs baseline: 1.5908x; 1.5908x over previous
"""Trainium2 Bass kernel for an 8-expert top-2 MoE layer.

Problem (hardcoded): x[8,2048,1024] f32, gate Wg[1024,8]+bg, experts
W1[8,1024,2048]+b1, W2[8,2048,1024]+b2, top-2 routing with renormalized
gate weights, out[8,2048,1024] f32.

Strategy: data-parallel over tokens. Each of the 8 NeuronCores processes one
batch row (2048 tokens) with all experts resident:
  1. gate logits via PE (fp32), top-2 + weights via DVE max8,
  2. build per-expert token lists on-device (one-hot transpose -> free-axis
     cumsum -> positions -> indirect scatter of token ids),
  3. per expert: dma_gather(transpose) dispatches routed tokens into a
     [D,tok] bf16 activation panel; two bf16 matmuls (weights stationary as
     lhsT) with fused bias+ReLU eviction; xbar DMA-transpose back to
     token-major; linear store into a [expert-slot, D] bf16 workspace,
  4. final combine: per token dma_gather of its two expert rows, scale by
     gate weights in fp32, store.
The capacity per (core, expert) is CAP=640 slots (mean load 512); overflow
beyond CAP is clamped into an unprocessed spill slot (probability ~0 for
gaussian inputs).
"""

import sys

for _p in ("/opt/trn_rl_repo",):
    if _p not in sys.path:
        sys.path.append(_p)

import numpy as np
import ml_dtypes

import concourse.bass as bass
import concourse.bacc as bacc
import concourse.tile as tile
import concourse.mybir as mybir
from concourse.masks import make_identity

P = 128
B, S, D = 8, 2048, 1024
E, H, TOPK = 8, 2048, 2
T = S  # tokens per core (one batch row per core)
NT = T // P  # 16 token tiles
KD = D // P  # 8 contraction tiles for D
KH = H // P  # 16 contraction tiles for H
MH = H // P  # 16 output tiles for H
MD = D // P  # 8 output tiles for D
CAP = 640  # processed slots per (core, expert)
STRIDE = 656  # idxlist/ybuf row stride per expert (CAP + spill)
XROWS = T + 16  # xb pad rows; row T is the all-zero dump row
YROWS = E * STRIDE
NCH = ((0, 512), (512, 128))  # token chunks of CAP for PSUM banks
DT = mybir.dt


def build_program():
    nc = bacc.Bacc("TRN2", target_bir_lowering=False, debug=False, num_devices=8)

    xt = nc.dram_tensor("xt", [D, T], DT.float32, kind="ExternalInput").ap()
    xb = nc.dram_tensor("xb", [XROWS, D], DT.bfloat16, kind="ExternalInput").ap()
    wg = nc.dram_tensor("wg", [P, KD * E], DT.float32, kind="ExternalInput").ap()
    bgb = nc.dram_tensor("bgb", [P, E], DT.float32, kind="ExternalInput").ap()
    iotae = nc.dram_tensor("iotae", [P, E], DT.float32, kind="ExternalInput").ap()
    tokid = nc.dram_tensor("tokid", [P, NT], DT.int16, kind="ExternalInput").ap()
    repm = nc.dram_tensor("repm", [16, P], DT.float32, kind="ExternalInput").ap()
    w1l = nc.dram_tensor("w1l", [E, P, KD * H], DT.bfloat16, kind="ExternalInput").ap()
    w2l = nc.dram_tensor("w2l", [E, P, KH * D], DT.bfloat16, kind="ExternalInput").ap()
    b1l = nc.dram_tensor("b1l", [E, P, MH], DT.float32, kind="ExternalInput").ap()
    b2l = nc.dram_tensor("b2l", [E, P, MD], DT.float32, kind="ExternalInput").ap()
    out = nc.dram_tensor("out", [T, D], DT.float32, kind="ExternalOutput").ap()

    idxlist = nc.dram_tensor("idxlist", [YROWS, 1], DT.int16).ap()
    gbuf = nc.dram_tensor("gbuf", [2, T], DT.int16).ap()
    ybuf = nc.dram_tensor("ybuf", [YROWS, D], DT.bfloat16).ap()

    with tile.TileContext(nc) as tc, tc.tile_pool(name="pers", bufs=1) as pers:
        with (
            tc.tile_pool(name="route", bufs=3) as route,
            tc.tile_pool(name="routeps", bufs=2, space="PSUM") as routeps,
            tc.tile_pool(name="gateps", bufs=2, space="PSUM") as gateps,
            tc.tile_pool(name="xtp", bufs=3) as xtp,
        ):
            ident = pers.tile([P, P], DT.float32)
            make_identity(nc, ident)
            wg_sb = pers.tile([P, KD * E], DT.float32)
            nc.sync.dma_start(wg_sb[:], wg[:, :])
            bgb_sb = pers.tile([P, E], DT.float32)
            nc.sync.dma_start(bgb_sb[:], bgb[:, :])
            iot_sb = pers.tile([P, E], DT.float32)
            nc.sync.dma_start(iot_sb[:], iotae[:, :])
            tok_sb = pers.tile([P, NT], DT.int16)
            nc.sync.dma_start(tok_sb[:], tokid[:, :])
            rep_sb = pers.tile([16, P], DT.float32)
            nc.sync.dma_start(rep_sb[:], repm[:, :])

            # init idxlist to the dump token id (T -> zero row of xb)
            init_t = pers.tile([P, YROWS // P], DT.int16)
            nc.vector.memset(init_t[:], T)
            nc.sync.dma_start(idxlist.rearrange("(p c) o -> p (c o)", p=P), init_t[:])

            # zero the spill rows of ybuf (rows e*STRIDE+CAP .. e*STRIDE+655)
            zspill = pers.tile([16, D], DT.bfloat16)
            nc.vector.memset(zspill[:], 0)
            for e in range(E):
                nc.sync.dma_start(
                    ybuf[e * STRIDE + CAP : e * STRIDE + STRIDE, :], zspill[:]
                )

            combT = pers.tile([E, T], DT.float32)
            zerosE = pers.tile([E, T], DT.float32)
            nc.vector.memset(zerosE[:], 0.0)
            i1a = pers.tile([P, NT], DT.float32)
            i2a = pers.tile([P, NT], DT.float32)
            w1a = pers.tile([P, NT], DT.float32)
            w2a = pers.tile([P, NT], DT.float32)

            # ---- gate + top-2 per token tile ----
            for i in range(NT):
                xt_sb = xtp.tile([P, KD, P], DT.float32)
                nc.sync.dma_start(
                    xt_sb[:],
                    xt.rearrange("(k p) t -> p k t", p=P)[
                        :, :, i * P : (i + 1) * P
                    ],
                )
                ps_g = gateps.tile([P, E], DT.float32)
                for k in range(KD):
                    nc.tensor.matmul(
                        ps_g[:],
                        lhsT=xt_sb[:, k, :],
                        rhs=wg_sb[:, k * E : (k + 1) * E],
                        start=(k == 0),
                        stop=(k == KD - 1),
                    )
                logits = route.tile([P, E], DT.float32)
                nc.vector.tensor_add(logits[:], ps_g[:], bgb_sb[:])

                vals8 = route.tile([P, 8], DT.float32)
                idx8 = route.tile([P, 8], DT.uint32)
                nc.vector.max_with_indices(vals8[:], idx8[:], logits[:])
                nc.vector.tensor_copy(i1a[:, i : i + 1], idx8[:, 0:1])
                nc.vector.tensor_copy(i2a[:, i : i + 1], idx8[:, 1:2])

                oh1 = route.tile([P, E], DT.float32)
                nc.vector.tensor_scalar(
                    oh1[:], iot_sb[:], i1a[:, i : i + 1], None,
                    op0=mybir.AluOpType.is_equal,
                )
                oh2 = route.tile([P, E], DT.float32)
                nc.vector.tensor_scalar(
                    oh2[:], iot_sb[:], i2a[:, i : i + 1], None,
                    op0=mybir.AluOpType.is_equal,
                )
                comb = route.tile([P, E], DT.float32)
                nc.vector.tensor_add(comb[:], oh1[:], oh2[:])
                ps_t = routeps.tile([E, P], DT.float32)
                nc.tensor.transpose(ps_t[:], comb[:, :], ident[:])
                nc.vector.tensor_copy(combT[:, i * P : (i + 1) * P], ps_t[:])

                dm = route.tile([P, 1], DT.float32)
                nc.vector.tensor_sub(dm[:], vals8[:, 1:2], vals8[:, 0:1])
                ed = route.tile([P, 1], DT.float32)
                nc.scalar.activation(ed[:], dm[:], mybir.ActivationFunctionType.Exp)
                den = route.tile([P, 1], DT.float32)
                nc.vector.tensor_scalar_add(den[:], ed[:], 1.0)
                nc.vector.reciprocal(w1a[:, i : i + 1], den[:])
                nc.vector.tensor_mul(w2a[:, i : i + 1], ed[:], w1a[:, i : i + 1])

            # ---- cumulative per-expert counts -> slot positions ----
            incl = pers.tile([E, T], DT.float32)
            nc.vector.tensor_tensor_scan(
                incl[:], combT[:], zerosE[:], 0.0,
                op0=mybir.AluOpType.add, op1=mybir.AluOpType.add,
            )
            excl = pers.tile([E, T], DT.float32)
            nc.vector.tensor_sub(excl[:], incl[:], combT[:])

            for i in range(NT):
                ps_e = routeps.tile([P, E], DT.float32)
                nc.tensor.transpose(
                    ps_e[:], excl[:, i * P : (i + 1) * P], ident[0:E, 0:E]
                )
                excl_tok = route.tile([P, E], DT.float32)
                nc.vector.tensor_copy(excl_tok[:], ps_e[:])
                for slot, ifc in ((0, i1a), (1, i2a)):
                    oh = route.tile([P, E], DT.float32)
                    nc.vector.tensor_scalar(
                        oh[:], iot_sb[:], ifc[:, i : i + 1], None,
                        op0=mybir.AluOpType.is_equal,
                    )
                    tmp = route.tile([P, E], DT.float32)
                    nc.vector.tensor_mul(tmp[:], excl_tok[:], oh[:])
                    ppos = route.tile([P, 1], DT.float32)
                    nc.vector.tensor_reduce(
                        ppos[:], tmp[:], axis=mybir.AxisListType.X,
                        op=mybir.AluOpType.add,
                    )
                    pm = route.tile([P, 1], DT.float32)
                    nc.vector.tensor_scalar_min(pm[:], ppos[:], float(CAP))
                    g = route.tile([P, 1], DT.float32)
                    nc.vector.tensor_scalar(
                        g[:], ifc[:, i : i + 1], float(STRIDE), pm[:, 0:1],
                        op0=mybir.AluOpType.mult, op1=mybir.AluOpType.add,
                    )
                    gi = route.tile([P, 1], DT.int32)
                    nc.vector.tensor_copy(gi[:], g[:])
                    nc.gpsimd.indirect_dma_start(
                        out=idxlist,
                        out_offset=bass.IndirectOffsetOnAxis(ap=gi[:, 0:1], axis=0),
                        in_=tok_sb[:, i : i + 1],
                        in_offset=None,
                    )
                    gs = route.tile([P, 1], DT.int16)
                    nc.vector.tensor_copy(gs[:], g[:])
                    nc.sync.dma_start(
                        gbuf[slot : slot + 1, i * P : (i + 1) * P], gs[:, 0:1]
                    )

        # ---- replicated wrapped idx panel (one per kernel) ----
        # idxrep[q, e*41+c] = idxlist[e*656 + c*16 + (q%16)]; partition-group
        # replication done on PE via the 0/1 matrix rep_sb.
        with tc.tile_pool(name="idxrep_ps", bufs=1, space="PSUM") as irps:
            idxw = pers.tile([16, E * 41], DT.int16)
            with nc.allow_non_contiguous_dma(reason="wrapped idx load, 10KB once"):
                nc.sync.dma_start(
                    idxw[:, :],
                    idxlist.rearrange("(e c p) o -> p (e c o)", p=16, c=41),
                )
            idxwf = pers.tile([16, E * 41], DT.float32)
            nc.vector.tensor_copy(idxwf[:], idxw[:])
            ip1 = irps.tile([P, 164], DT.float32)
            nc.tensor.matmul(ip1[:], lhsT=rep_sb[:], rhs=idxwf[:, 0:164],
                             start=True, stop=True)
            ip2 = irps.tile([P, 164], DT.float32)
            nc.tensor.matmul(ip2[:], lhsT=rep_sb[:], rhs=idxwf[:, 164:328],
                             start=True, stop=True)
            idxrep = pers.tile([P, E * 41], DT.int16)
            nc.vector.tensor_copy(idxrep[:, 0:164], ip1[:])
            nc.vector.tensor_copy(idxrep[:, 164:328], ip2[:])

        # ---- expert loop ----
        with (
            tc.tile_pool(name="w1p", bufs=2) as w1p,
            tc.tile_pool(name="w2p", bufs=2) as w2p,
            tc.tile_pool(name="bp", bufs=2) as bp,
            tc.tile_pool(name="xg", bufs=2) as xg,
            tc.tile_pool(name="hp", bufs=1) as hp,
            tc.tile_pool(name="yp", bufs=2) as yp,
            tc.tile_pool(name="ytk", bufs=2) as ytk,
            tc.tile_pool(name="mm1ps", bufs=3, space="PSUM") as mm1ps,
            tc.tile_pool(name="mm2ps", bufs=3, space="PSUM") as mm2ps,
        ):
            for e in range(E):
                xgT = xg.tile([P, KD, CAP], DT.bfloat16)
                nc.gpsimd.dma_gather(
                    out_ap=xgT[:],
                    in_ap=xb[:, :],
                    idxs_ap=idxrep[:, e * 41 : e * 41 + CAP // 16],
                    num_idxs=CAP,
                    num_idxs_reg=CAP,
                    elem_size=D,
                    transpose=True,
                )

                w1t = []
                for t in range(2):
                    wt = w1p.tile([P, 4 * H], DT.bfloat16, tag="w1")
                    nc.sync.dma_start(wt[:], w1l[e, :, t * 4 * H : (t + 1) * 4 * H])
                    w1t.append(wt)
                w2t = []
                for t in range(2):
                    wt = w2p.tile([P, 8 * D], DT.bfloat16, tag="w2")
                    nc.sync.dma_start(wt[:], w2l[e, :, t * 8 * D : (t + 1) * 8 * D])
                    w2t.append(wt)
                w1k = [w1t[k // 4][:, (k % 4) * H : (k % 4) * H + H] for k in range(KD)]
                w2k = [w2t[k // 8][:, (k % 8) * D : (k % 8) * D + D] for k in range(KH)]
                b1t = bp.tile([P, MH], DT.float32, tag="b1")
                nc.sync.dma_start(b1t[:], b1l[e, :, :])
                b2t = bp.tile([P, MD], DT.float32, tag="b2")
                nc.sync.dma_start(b2t[:], b2l[e, :, :])

                hT = [
                    hp.tile([P, CAP], DT.bfloat16, tag=f"hT{m}", name=f"hT{m}")
                    for m in range(MH)
                ]
                for m in range(MH):
                    for n0, nsz in NCH:
                        ps1 = mm1ps.tile([P, 512], DT.float32)
                        for k in range(KD):
                            nc.tensor.matmul(
                                ps1[:, :nsz],
                                lhsT=w1k[k][:, m * P : (m + 1) * P],
                                rhs=xgT[:, k, n0 : n0 + nsz],
                                start=(k == 0),
                                stop=(k == KD - 1),
                            )
                        nc.scalar.activation(
                            hT[m][:, n0 : n0 + nsz],
                            ps1[:, :nsz],
                            mybir.ActivationFunctionType.Relu,
                            bias=b1t[:, m : m + 1],
                        )

                yT = [
                    yp.tile([P, CAP], DT.bfloat16, tag=f"yT{md}", name=f"yT{md}")
                    for md in range(MD)
                ]
                for md in range(MD):
                    for n0, nsz in NCH:
                        ps2 = mm2ps.tile([P, 512], DT.float32)
                        for k in range(KH):
                            nc.tensor.matmul(
                                ps2[:, :nsz],
                                lhsT=w2k[k][:, md * P : (md + 1) * P],
                                rhs=hT[k][:, n0 : n0 + nsz],
                                start=(k == 0),
                                stop=(k == KH - 1),
                            )
                        nc.scalar.activation(
                            yT[md][:, n0 : n0 + nsz],
                            ps2[:, :nsz],
                            mybir.ActivationFunctionType.Identity,
                            bias=b2t[:, md : md + 1],
                        )

                ytok = ytk.tile([P, CAP // P, D], DT.bfloat16)
                for md in range(MD):
                    nc.sync.dma_start_transpose(
                        ytok[:, :, md * P : (md + 1) * P],
                        yT[md][:, :],
                    )
                nc.sync.dma_start(
                    ybuf[e * STRIDE : e * STRIDE + CAP, :].rearrange(
                        "(c p) d -> p c d", p=P
                    ),
                    ytok[:],
                )

        # ---- final combine ----
        with (
            tc.tile_pool(name="fin", bufs=4) as fin,
            tc.tile_pool(name="fing", bufs=1) as fing,
            tc.tile_pool(name="finps", bufs=2, space="PSUM") as finps,
        ):
            gth = []
            for slot in range(2):
                gw = fin.tile([16, NT * 8], DT.int16, tag="gw")
                with nc.allow_non_contiguous_dma(reason="wrapped idx load, 4KB once"):
                    nc.sync.dma_start(
                        gw[:, :],
                        gbuf[slot, :].rearrange("(c p) -> p c", p=16),
                    )
                gwf = fin.tile([16, NT * 8], DT.float32, tag="gwf")
                nc.vector.tensor_copy(gwf[:], gw[:])
                gps = finps.tile([P, NT * 8], DT.float32)
                nc.tensor.matmul(gps[:], lhsT=rep_sb[:], rhs=gwf[:],
                                 start=True, stop=True)
                grep = fin.tile([P, NT * 8], DT.int16, tag="grep")
                nc.vector.tensor_copy(grep[:], gps[:])
                # dma_gather dies above 512 idxs (non-transpose mode), so
                # fetch in 512-token chunks
                gts = []
                for ch in range(NT // 4):
                    gt = fing.tile([P, 4, D], DT.bfloat16, tag=f"gth{slot}_{ch}",
                                   name=f"gth{slot}_{ch}")
                    nc.gpsimd.dma_gather(
                        out_ap=gt[:],
                        in_ap=ybuf[:, :],
                        idxs_ap=grep[:, ch * 32 : (ch + 1) * 32],
                        num_idxs=512,
                        num_idxs_reg=512,
                        elem_size=D,
                    )
                    gts.append(gt)
                gth.append(gts)
            for i in range(NT):
                acc1 = fin.tile([P, D], DT.float32, tag="acc1")
                nc.vector.tensor_scalar(
                    acc1[:], gth[0][i // 4][:, i % 4, :], w1a[:, i : i + 1], None,
                    op0=mybir.AluOpType.mult,
                )
                acc2 = fin.tile([P, D], DT.float32, tag="acc2")
                nc.vector.tensor_scalar(
                    acc2[:], gth[1][i // 4][:, i % 4, :], w2a[:, i : i + 1], None,
                    op0=mybir.AluOpType.mult,
                )
                res = fin.tile([P, D], DT.float32, tag="res")
                nc.vector.tensor_add(res[:], acc1[:], acc2[:])
                nc.sync.dma_start(out[i * P : (i + 1) * P, :], res[:])

    nc.compile()
    return nc


def prep_inputs(x, Wg, bg, W1, b1, W2, b2):
    """Build the 8 per-core input maps from full problem inputs (numpy f32)."""
    bf16 = ml_dtypes.bfloat16
    wg_l = np.ascontiguousarray(
        Wg.reshape(KD, P, E).transpose(1, 0, 2).reshape(P, KD * E)
    )
    bgb_np = np.tile(bg[None, :], (P, 1)).astype(np.float32)
    iot_np = np.tile(np.arange(E, dtype=np.float32)[None, :], (P, 1))
    tok_np = np.arange(T, dtype=np.int16).reshape(NT, P).T.copy()
    rep_np = (np.arange(P)[None, :] % 16 == np.arange(16)[:, None]).astype(np.float32)
    w1l_np = np.ascontiguousarray(
        W1.reshape(E, KD, P, H).transpose(0, 2, 1, 3).reshape(E, P, KD * H)
    ).astype(bf16)
    w2l_np = np.ascontiguousarray(
        W2.reshape(E, KH, P, D).transpose(0, 2, 1, 3).reshape(E, P, KH * D)
    ).astype(bf16)
    b1l_np = np.ascontiguousarray(
        b1.reshape(E, MH, P).transpose(0, 2, 1)
    ).astype(np.float32)
    b2l_np = np.ascontiguousarray(
        b2.reshape(E, MD, P).transpose(0, 2, 1)
    ).astype(np.float32)

    in_maps = []
    for c in range(B):
        xc = np.asarray(x[c], dtype=np.float32)  # [T, D]
        xt_np = np.ascontiguousarray(xc.T)
        xb_np = np.zeros((XROWS, D), dtype=bf16)
        xb_np[:T] = xc.astype(bf16)
        in_maps.append(
            {
                "xt": xt_np,
                "xb": xb_np,
                "wg": wg_l,
                "bgb": bgb_np,
                "iotae": iot_np,
                "tokid": tok_np,
                "repm": rep_np,
                "w1l": w1l_np,
                "w2l": w2l_np,
                "b1l": b1l_np,
                "b2l": b2l_np,
            }
        )
    return in_maps


_nc_cache = None


def kernel(**inputs):
    global _nc_cache
    from concourse.bass_utils import run_bass_kernel_spmd

    if _nc_cache is None:
        _nc_cache = build_program()
    nc = _nc_cache
    in_maps = prep_inputs(
        np.asarray(inputs["x"], dtype=np.float32),
        np.asarray(inputs["Wg"], dtype=np.float32),
        np.asarray(inputs["bg"], dtype=np.float32),
        np.asarray(inputs["W1"], dtype=np.float32),
        np.asarray(inputs["b1"], dtype=np.float32),
        np.asarray(inputs["W2"], dtype=np.float32),
        np.asarray(inputs["b2"], dtype=np.float32),
    )
    res = run_bass_kernel_spmd(nc, in_maps, core_ids=list(range(B)))
    out = np.stack([res.results[c]["out"] for c in range(B)], axis=0)
    return out.astype(np.float32)


# revision 10
# speedup vs baseline: 1.9025x; 1.1959x over previous
"""Trainium2 Bass kernel for an 8-expert top-2 MoE layer.

Problem (hardcoded): x[8,2048,1024] f32, gate Wg[1024,8]+bg, experts
W1[8,1024,2048]+b1, W2[8,2048,1024]+b2, top-2 routing with renormalized
gate weights, out[8,2048,1024] f32.

Strategy: data-parallel over tokens. Each of the 8 NeuronCores processes one
batch row (2048 tokens) with all experts resident:
  1. gate logits via PE (fp32), top-2 + weights via DVE max8,
  2. build per-expert token lists on-device (one-hot transpose -> free-axis
     cumsum -> positions -> indirect scatter of token ids),
  3. per expert: dma_gather(transpose) dispatches routed tokens into a
     [D,tok] bf16 activation panel; two bf16 matmuls (weights stationary as
     lhsT) with fused bias+ReLU eviction; xbar DMA-transpose back to
     token-major; linear store into a [expert-slot, D] bf16 workspace,
  4. final combine: per token dma_gather of its two expert rows, scale by
     gate weights in fp32, store.
The capacity per (core, expert) is CAP=640 slots (mean load 512); overflow
beyond CAP is clamped into an unprocessed spill slot (probability ~0 for
gaussian inputs).
"""

import sys

for _p in ("/opt/trn_rl_repo",):
    if _p not in sys.path:
        sys.path.append(_p)

import numpy as np
import ml_dtypes

import concourse.bass as bass
import concourse.bacc as bacc
import concourse.tile as tile
import concourse.mybir as mybir
from concourse.masks import make_identity

P = 128
B, S, D = 8, 2048, 1024
E, H, TOPK = 8, 2048, 2
T = S  # tokens per core (one batch row per core)
NT = T // P  # 16 token tiles
KD = D // P  # 8 contraction tiles for D
KH = H // P  # 16 contraction tiles for H
MH = H // P  # 16 output tiles for H
MD = D // P  # 8 output tiles for D
CAP = 640  # processed slots per (core, expert)
STRIDE = 656  # idxlist/ybuf row stride per expert (CAP + spill)
XROWS = T + 16  # xb pad rows; row T is the all-zero dump row
YROWS = E * STRIDE
NCH = ((0, 512), (512, 128))  # token chunks of CAP for PSUM banks
DT = mybir.dt


def build_program():
    nc = bacc.Bacc("TRN2", target_bir_lowering=False, debug=False, num_devices=8)

    xt = nc.dram_tensor("xt", [D, T], DT.float32, kind="ExternalInput").ap()
    xb = nc.dram_tensor("xb", [XROWS, D], DT.bfloat16, kind="ExternalInput").ap()
    wg = nc.dram_tensor("wg", [P, KD * E], DT.float32, kind="ExternalInput").ap()
    bgb = nc.dram_tensor("bgb", [P, E], DT.float32, kind="ExternalInput").ap()
    iotae = nc.dram_tensor("iotae", [P, E], DT.float32, kind="ExternalInput").ap()
    tokid = nc.dram_tensor("tokid", [P, NT], DT.int16, kind="ExternalInput").ap()
    repm = nc.dram_tensor("repm", [16, P], DT.float32, kind="ExternalInput").ap()
    w1l = nc.dram_tensor("w1l", [E, P, KD * H], DT.bfloat16, kind="ExternalInput").ap()
    w2l = nc.dram_tensor("w2l", [E, P, KH * D], DT.bfloat16, kind="ExternalInput").ap()
    b1l = nc.dram_tensor("b1l", [E, P, MH], DT.float32, kind="ExternalInput").ap()
    b2l = nc.dram_tensor("b2l", [E, P, MD], DT.float32, kind="ExternalInput").ap()
    out = nc.dram_tensor("out", [T, D], DT.float32, kind="ExternalOutput").ap()

    idxlist = nc.dram_tensor("idxlist", [YROWS, 1], DT.int16).ap()
    gbuf = nc.dram_tensor("gbuf", [2, T], DT.int16).ap()
    ybuf = nc.dram_tensor("ybuf", [YROWS, D], DT.bfloat16).ap()

    with tile.TileContext(nc) as tc, tc.tile_pool(name="pers", bufs=1) as pers:
        with (
            tc.tile_pool(name="route", bufs=24) as route,
            tc.tile_pool(name="routeps", bufs=2, space="PSUM") as routeps,
            tc.tile_pool(name="gateps", bufs=2, space="PSUM") as gateps,
            tc.tile_pool(name="xtp", bufs=5) as xtp,
        ):
            ident = pers.tile([P, P], DT.float32)
            make_identity(nc, ident)
            wg_sb = pers.tile([P, KD * E], DT.float32)
            nc.sync.dma_start(wg_sb[:], wg[:, :])
            bgb_sb = pers.tile([P, E], DT.float32)
            nc.sync.dma_start(bgb_sb[:], bgb[:, :])
            iot_sb = pers.tile([P, E], DT.float32)
            nc.sync.dma_start(iot_sb[:], iotae[:, :])
            tok_sb = pers.tile([P, NT], DT.int16)
            nc.sync.dma_start(tok_sb[:], tokid[:, :])
            rep_sb = pers.tile([16, P], DT.float32)
            nc.sync.dma_start(rep_sb[:], repm[:, :])

            # init idxlist to the dump token id (T -> zero row of xb)
            init_t = pers.tile([P, YROWS // P], DT.int16)
            nc.vector.memset(init_t[:], T)
            nc.sync.dma_start(idxlist.rearrange("(p c) o -> p (c o)", p=P), init_t[:])

            # zero the spill rows of ybuf (rows e*STRIDE+CAP .. e*STRIDE+655)
            zspill = pers.tile([16, D], DT.bfloat16)
            nc.vector.memset(zspill[:], 0)
            for e in range(E):
                nc.sync.dma_start(
                    ybuf[e * STRIDE + CAP : e * STRIDE + STRIDE, :], zspill[:]
                )

            combT = pers.tile([E, T], DT.float32)
            zerosE = pers.tile([E, T], DT.float32)
            nc.vector.memset(zerosE[:], 0.0)
            i1a = pers.tile([P, NT], DT.float32)
            i2a = pers.tile([P, NT], DT.float32)
            w1a = pers.tile([P, NT], DT.float32)
            w2a = pers.tile([P, NT], DT.float32)

            # ---- gate + top-2 per token tile ----
            for i in range(NT):
                xt_sb = xtp.tile([P, KD, P], DT.float32)
                nc.sync.dma_start(
                    xt_sb[:],
                    xt.rearrange("(k p) t -> p k t", p=P)[
                        :, :, i * P : (i + 1) * P
                    ],
                )
                ps_g = gateps.tile([P, E], DT.float32)
                for k in range(KD):
                    nc.tensor.matmul(
                        ps_g[:],
                        lhsT=xt_sb[:, k, :],
                        rhs=wg_sb[:, k * E : (k + 1) * E],
                        start=(k == 0),
                        stop=(k == KD - 1),
                    )
                logits = route.tile([P, E], DT.float32)
                nc.vector.tensor_add(logits[:], ps_g[:], bgb_sb[:])

                vals8 = route.tile([P, 8], DT.float32)
                idx8 = route.tile([P, 8], DT.uint32)
                nc.vector.max_with_indices(vals8[:], idx8[:], logits[:])
                nc.vector.tensor_copy(i1a[:, i : i + 1], idx8[:, 0:1])
                nc.vector.tensor_copy(i2a[:, i : i + 1], idx8[:, 1:2])

                oh1 = route.tile([P, E], DT.float32)
                nc.vector.tensor_scalar(
                    oh1[:], iot_sb[:], i1a[:, i : i + 1], None,
                    op0=mybir.AluOpType.is_equal,
                )
                oh2 = route.tile([P, E], DT.float32)
                nc.vector.tensor_scalar(
                    oh2[:], iot_sb[:], i2a[:, i : i + 1], None,
                    op0=mybir.AluOpType.is_equal,
                )
                comb = route.tile([P, E], DT.float32)
                nc.vector.tensor_add(comb[:], oh1[:], oh2[:])
                ps_t = routeps.tile([E, P], DT.float32)
                nc.tensor.transpose(ps_t[:], comb[:, :], ident[:])
                nc.vector.tensor_copy(combT[:, i * P : (i + 1) * P], ps_t[:])

                dm = route.tile([P, 1], DT.float32)
                nc.vector.tensor_sub(dm[:], vals8[:, 1:2], vals8[:, 0:1])
                ed = route.tile([P, 1], DT.float32)
                nc.scalar.activation(ed[:], dm[:], mybir.ActivationFunctionType.Exp)
                den = route.tile([P, 1], DT.float32)
                nc.vector.tensor_scalar_add(den[:], ed[:], 1.0)
                nc.vector.reciprocal(w1a[:, i : i + 1], den[:])
                nc.vector.tensor_mul(w2a[:, i : i + 1], ed[:], w1a[:, i : i + 1])

            # ---- cumulative per-expert counts -> slot positions ----
            incl = pers.tile([E, T], DT.float32)
            nc.vector.tensor_tensor_scan(
                incl[:], combT[:], zerosE[:], 0.0,
                op0=mybir.AluOpType.add, op1=mybir.AluOpType.add,
            )
            excl = pers.tile([E, T], DT.float32)
            nc.vector.tensor_sub(excl[:], incl[:], combT[:])

            for i in range(NT):
                ps_e = routeps.tile([P, E], DT.float32)
                nc.tensor.transpose(
                    ps_e[:], excl[:, i * P : (i + 1) * P], ident[0:E, 0:E]
                )
                excl_tok = route.tile([P, E], DT.float32)
                nc.vector.tensor_copy(excl_tok[:], ps_e[:])
                for slot, ifc in ((0, i1a), (1, i2a)):
                    oh = route.tile([P, E], DT.float32)
                    nc.vector.tensor_scalar(
                        oh[:], iot_sb[:], ifc[:, i : i + 1], None,
                        op0=mybir.AluOpType.is_equal,
                    )
                    tmp = route.tile([P, E], DT.float32)
                    nc.vector.tensor_mul(tmp[:], excl_tok[:], oh[:])
                    ppos = route.tile([P, 1], DT.float32)
                    nc.vector.tensor_reduce(
                        ppos[:], tmp[:], axis=mybir.AxisListType.X,
                        op=mybir.AluOpType.add,
                    )
                    pm = route.tile([P, 1], DT.float32)
                    nc.vector.tensor_scalar_min(pm[:], ppos[:], float(CAP))
                    g = route.tile([P, 1], DT.float32)
                    nc.vector.tensor_scalar(
                        g[:], ifc[:, i : i + 1], float(STRIDE), pm[:, 0:1],
                        op0=mybir.AluOpType.mult, op1=mybir.AluOpType.add,
                    )
                    gi = route.tile([P, 1], DT.int32)
                    nc.vector.tensor_copy(gi[:], g[:])
                    nc.gpsimd.indirect_dma_start(
                        out=idxlist,
                        out_offset=bass.IndirectOffsetOnAxis(ap=gi[:, 0:1], axis=0),
                        in_=tok_sb[:, i : i + 1],
                        in_offset=None,
                    )
                    gs = route.tile([P, 1], DT.int16)
                    nc.vector.tensor_copy(gs[:], g[:])
                    nc.sync.dma_start(
                        gbuf[slot : slot + 1, i * P : (i + 1) * P], gs[:, 0:1]
                    )

        # ---- replicated wrapped idx panel (one per kernel) ----
        # idxrep[q, e*41+c] = idxlist[e*656 + c*16 + (q%16)]; partition-group
        # replication done on PE via the 0/1 matrix rep_sb.
        with tc.tile_pool(name="idxrep_ps", bufs=1, space="PSUM") as irps:
            idxw = pers.tile([16, E * 41], DT.int16)
            with nc.allow_non_contiguous_dma(reason="wrapped idx load, 10KB once"):
                nc.sync.dma_start(
                    idxw[:, :],
                    idxlist.rearrange("(e c p) o -> p (e c o)", p=16, c=41),
                )
            idxwf = pers.tile([16, E * 41], DT.float32)
            nc.vector.tensor_copy(idxwf[:], idxw[:])
            ip1 = irps.tile([P, 164], DT.float32)
            nc.tensor.matmul(ip1[:], lhsT=rep_sb[:], rhs=idxwf[:, 0:164],
                             start=True, stop=True)
            ip2 = irps.tile([P, 164], DT.float32)
            nc.tensor.matmul(ip2[:], lhsT=rep_sb[:], rhs=idxwf[:, 164:328],
                             start=True, stop=True)
            idxrep = pers.tile([P, E * 41], DT.int16)
            nc.vector.tensor_copy(idxrep[:, 0:164], ip1[:])
            nc.vector.tensor_copy(idxrep[:, 164:328], ip2[:])

        # ---- expert loop ----
        with (
            tc.tile_pool(name="w1p", bufs=2) as w1p,
            tc.tile_pool(name="w2p", bufs=2) as w2p,
            tc.tile_pool(name="bp", bufs=2) as bp,
            tc.tile_pool(name="xg", bufs=2) as xg,
            tc.tile_pool(name="hp", bufs=1) as hp,
            tc.tile_pool(name="yp", bufs=2) as yp,
            tc.tile_pool(name="ytk", bufs=2) as ytk,
            tc.tile_pool(name="mm1ps", bufs=3, space="PSUM") as mm1ps,
            tc.tile_pool(name="mm2ps", bufs=3, space="PSUM") as mm2ps,
        ):
            for e in range(E):
                xgT = xg.tile([P, KD, CAP], DT.bfloat16)
                nc.gpsimd.dma_gather(
                    out_ap=xgT[:],
                    in_ap=xb[:, :],
                    idxs_ap=idxrep[:, e * 41 : e * 41 + CAP // 16],
                    num_idxs=CAP,
                    num_idxs_reg=CAP,
                    elem_size=D,
                    transpose=True,
                )

                w1t = []
                for t in range(2):
                    wt = w1p.tile([P, 4 * H], DT.bfloat16, tag="w1")
                    nc.scalar.dma_start(wt[:], w1l[e, :, t * 4 * H : (t + 1) * 4 * H])
                    w1t.append(wt)
                w2t = []
                for t in range(2):
                    wt = w2p.tile([P, 8 * D], DT.bfloat16, tag="w2")
                    nc.scalar.dma_start(wt[:], w2l[e, :, t * 8 * D : (t + 1) * 8 * D])
                    w2t.append(wt)
                w1k = [w1t[k // 4][:, (k % 4) * H : (k % 4) * H + H] for k in range(KD)]
                w2k = [w2t[k // 8][:, (k % 8) * D : (k % 8) * D + D] for k in range(KH)]
                b1t = bp.tile([P, MH], DT.float32, tag="b1")
                nc.scalar.dma_start(b1t[:], b1l[e, :, :])
                b2t = bp.tile([P, MD], DT.float32, tag="b2")
                nc.scalar.dma_start(b2t[:], b2l[e, :, :])

                hT = [
                    hp.tile([P, CAP], DT.bfloat16, tag=f"hT{m}", name=f"hT{m}")
                    for m in range(MH)
                ]
                for m in range(MH):
                    for n0, nsz in NCH:
                        ps1 = mm1ps.tile([P, 512], DT.float32)
                        for k in range(KD):
                            nc.tensor.matmul(
                                ps1[:, :nsz],
                                lhsT=w1k[k][:, m * P : (m + 1) * P],
                                rhs=xgT[:, k, n0 : n0 + nsz],
                                start=(k == 0),
                                stop=(k == KD - 1),
                            )
                        nc.scalar.activation(
                            hT[m][:, n0 : n0 + nsz],
                            ps1[:, :nsz],
                            mybir.ActivationFunctionType.Relu,
                            bias=b1t[:, m : m + 1],
                        )

                yT = [
                    yp.tile([P, CAP], DT.bfloat16, tag=f"yT{md}", name=f"yT{md}")
                    for md in range(MD)
                ]
                for md in range(MD):
                    for n0, nsz in NCH:
                        ps2 = mm2ps.tile([P, 512], DT.float32)
                        for k in range(KH):
                            nc.tensor.matmul(
                                ps2[:, :nsz],
                                lhsT=w2k[k][:, md * P : (md + 1) * P],
                                rhs=hT[k][:, n0 : n0 + nsz],
                                start=(k == 0),
                                stop=(k == KH - 1),
                            )
                        nc.scalar.activation(
                            yT[md][:, n0 : n0 + nsz],
                            ps2[:, :nsz],
                            mybir.ActivationFunctionType.Identity,
                            bias=b2t[:, md : md + 1],
                        )

                ytok = ytk.tile([P, CAP // P, D], DT.bfloat16)
                for md in range(MD):
                    nc.sync.dma_start_transpose(
                        ytok[:, :, md * P : (md + 1) * P],
                        yT[md][:, :],
                    )
                nc.sync.dma_start(
                    ybuf[e * STRIDE : e * STRIDE + CAP, :].rearrange(
                        "(c p) d -> p c d", p=P
                    ),
                    ytok[:],
                )

        # ---- final combine ----
        with (
            tc.tile_pool(name="fin", bufs=4) as fin,
            tc.tile_pool(name="fing", bufs=1) as fing,
            tc.tile_pool(name="finps", bufs=2, space="PSUM") as finps,
        ):
            gth = []
            for slot in range(2):
                gw = fin.tile([16, NT * 8], DT.int16, tag="gw")
                with nc.allow_non_contiguous_dma(reason="wrapped idx load, 4KB once"):
                    nc.sync.dma_start(
                        gw[:, :],
                        gbuf[slot, :].rearrange("(c p) -> p c", p=16),
                    )
                gwf = fin.tile([16, NT * 8], DT.float32, tag="gwf")
                nc.vector.tensor_copy(gwf[:], gw[:])
                gps = finps.tile([P, NT * 8], DT.float32)
                nc.tensor.matmul(gps[:], lhsT=rep_sb[:], rhs=gwf[:],
                                 start=True, stop=True)
                grep = fin.tile([P, NT * 8], DT.int16, tag="grep")
                nc.vector.tensor_copy(grep[:], gps[:])
                # dma_gather dies above 512 idxs (non-transpose mode), so
                # fetch in 512-token chunks
                gts = []
                for ch in range(NT // 4):
                    gt = fing.tile([P, 4, D], DT.bfloat16, tag=f"gth{slot}_{ch}",
                                   name=f"gth{slot}_{ch}")
                    nc.gpsimd.dma_gather(
                        out_ap=gt[:],
                        in_ap=ybuf[:, :],
                        idxs_ap=grep[:, ch * 32 : (ch + 1) * 32],
                        num_idxs=512,
                        num_idxs_reg=512,
                        elem_size=D,
                    )
                    gts.append(gt)
                gth.append(gts)
            for i in range(NT):
                acc1 = fin.tile([P, D], DT.float32, tag="acc1")
                nc.vector.tensor_scalar(
                    acc1[:], gth[0][i // 4][:, i % 4, :], w1a[:, i : i + 1], None,
                    op0=mybir.AluOpType.mult,
                )
                acc2 = fin.tile([P, D], DT.float32, tag="acc2")
                nc.vector.tensor_scalar(
                    acc2[:], gth[1][i // 4][:, i % 4, :], w2a[:, i : i + 1], None,
                    op0=mybir.AluOpType.mult,
                )
                res = fin.tile([P, D], DT.float32, tag="res")
                nc.vector.tensor_add(res[:], acc1[:], acc2[:])
                nc.sync.dma_start(out[i * P : (i + 1) * P, :], res[:])

    nc.compile()
    return nc


def prep_inputs(x, Wg, bg, W1, b1, W2, b2):
    """Build the 8 per-core input maps from full problem inputs (numpy f32)."""
    bf16 = ml_dtypes.bfloat16
    wg_l = np.ascontiguousarray(
        Wg.reshape(KD, P, E).transpose(1, 0, 2).reshape(P, KD * E)
    )
    bgb_np = np.tile(bg[None, :], (P, 1)).astype(np.float32)
    iot_np = np.tile(np.arange(E, dtype=np.float32)[None, :], (P, 1))
    tok_np = np.arange(T, dtype=np.int16).reshape(NT, P).T.copy()
    rep_np = (np.arange(P)[None, :] % 16 == np.arange(16)[:, None]).astype(np.float32)
    w1l_np = np.ascontiguousarray(
        W1.reshape(E, KD, P, H).transpose(0, 2, 1, 3).reshape(E, P, KD * H)
    ).astype(bf16)
    w2l_np = np.ascontiguousarray(
        W2.reshape(E, KH, P, D).transpose(0, 2, 1, 3).reshape(E, P, KH * D)
    ).astype(bf16)
    b1l_np = np.ascontiguousarray(
        b1.reshape(E, MH, P).transpose(0, 2, 1)
    ).astype(np.float32)
    b2l_np = np.ascontiguousarray(
        b2.reshape(E, MD, P).transpose(0, 2, 1)
    ).astype(np.float32)

    in_maps = []
    for c in range(B):
        xc = np.asarray(x[c], dtype=np.float32)  # [T, D]
        xt_np = np.ascontiguousarray(xc.T)
        xb_np = np.zeros((XROWS, D), dtype=bf16)
        xb_np[:T] = xc.astype(bf16)
        in_maps.append(
            {
                "xt": xt_np,
                "xb": xb_np,
                "wg": wg_l,
                "bgb": bgb_np,
                "iotae": iot_np,
                "tokid": tok_np,
                "repm": rep_np,
                "w1l": w1l_np,
                "w2l": w2l_np,
                "b1l": b1l_np,
                "b2l": b2l_np,
            }
        )
    return in_maps


_nc_cache = None


def kernel(**inputs):
    global _nc_cache
    from concourse.bass_utils import run_bass_kernel_spmd

    if _nc_cache is None:
        _nc_cache = build_program()
    nc = _nc_cache
    in_maps = prep_inputs(
        np.asarray(inputs["x"], dtype=np.float32),
        np.asarray(inputs["Wg"], dtype=np.float32),
        np.asarray(inputs["bg"], dtype=np.float32),
        np.asarray(inputs["W1"], dtype=np.float32),
        np.asarray(inputs["b1"], dtype=np.float32),
        np.asarray(inputs["W2"], dtype=np.float32),
        np.asarray(inputs["b2"], dtype=np.float32),
    )
    res = run_bass_kernel_spmd(nc, in_maps, core_ids=list(range(B)))
    out = np.stack([res.results[c]["out"] for c in range(B)], axis=0)
    return out.astype(np.float32)


# revision 11
# speedup vs baseline: 2.2430x; 1.1790x over previous
"""Trainium2 Bass kernel for an 8-expert top-2 MoE layer.

Problem (hardcoded): x[8,2048,1024] f32, gate Wg[1024,8]+bg, experts
W1[8,1024,2048]+b1, W2[8,2048,1024]+b2, top-2 routing with renormalized
gate weights, out[8,2048,1024] f32.

Strategy: data-parallel over tokens. Each of the 8 NeuronCores processes one
batch row (2048 tokens) with all experts resident:
  1. gate logits via PE (fp32), top-2 + weights via DVE max8,
  2. build per-expert token lists on-device (one-hot transpose -> free-axis
     cumsum -> positions -> indirect scatter of token ids),
  3. per expert: dma_gather(transpose) dispatches routed tokens into a
     [D,tok] bf16 activation panel; two bf16 matmuls (weights stationary as
     lhsT) with fused bias+ReLU eviction; xbar DMA-transpose back to
     token-major; linear store into a [expert-slot, D] bf16 workspace,
  4. final combine: per token dma_gather of its two expert rows, scale by
     gate weights in fp32, store.
The capacity per (core, expert) is CAP=640 slots (mean load 512); overflow
beyond CAP is clamped into an unprocessed spill slot (probability ~0 for
gaussian inputs).
"""

import sys

for _p in ("/opt/trn_rl_repo",):
    if _p not in sys.path:
        sys.path.append(_p)

import numpy as np
import ml_dtypes

import concourse.bass as bass
import concourse.bacc as bacc
import concourse.tile as tile
import concourse.mybir as mybir
from concourse.masks import make_identity

P = 128
B, S, D = 8, 2048, 1024
E, H, TOPK = 8, 2048, 2
T = S  # tokens per core (one batch row per core)
NT = T // P  # 16 token tiles
KD = D // P  # 8 contraction tiles for D
KH = H // P  # 16 contraction tiles for H
MH = H // P  # 16 output tiles for H
MD = D // P  # 8 output tiles for D
CAP = 640  # processed slots per (core, expert)
STRIDE = 656  # idxlist/ybuf row stride per expert (CAP + spill)
XROWS = T + 16  # xb pad rows; row T is the all-zero dump row
YROWS = E * STRIDE
NCH = ((0, 512), (512, 128))  # token chunks of CAP for PSUM banks
DT = mybir.dt


def build_program():
    nc = bacc.Bacc("TRN2", target_bir_lowering=False, debug=False, num_devices=8)

    xt = nc.dram_tensor("xt", [D, T], DT.float32, kind="ExternalInput").ap()
    xb = nc.dram_tensor("xb", [XROWS, D], DT.bfloat16, kind="ExternalInput").ap()
    wg = nc.dram_tensor("wg", [P, KD * E], DT.float32, kind="ExternalInput").ap()
    bgb = nc.dram_tensor("bgb", [P, E], DT.float32, kind="ExternalInput").ap()
    iotae = nc.dram_tensor("iotae", [P, E], DT.float32, kind="ExternalInput").ap()
    tokide = nc.dram_tensor("tokide", [16, T], DT.int16, kind="ExternalInput").ap()
    repm = nc.dram_tensor("repm", [16, P], DT.float32, kind="ExternalInput").ap()
    w1l = nc.dram_tensor("w1l", [E, P, KD * H], DT.bfloat16, kind="ExternalInput").ap()
    w2l = nc.dram_tensor("w2l", [E, P, KH * D], DT.bfloat16, kind="ExternalInput").ap()
    b1l = nc.dram_tensor("b1l", [E, P, MH], DT.float32, kind="ExternalInput").ap()
    b2l = nc.dram_tensor("b2l", [E, P, MD], DT.float32, kind="ExternalInput").ap()
    out = nc.dram_tensor("out", [T, D], DT.float32, kind="ExternalOutput").ap()

    idxlist = nc.dram_tensor("idxlist", [YROWS, 1], DT.int16).ap()
    gbuf = nc.dram_tensor("gbuf", [2, T], DT.int16).ap()
    ybuf = nc.dram_tensor("ybuf", [YROWS, D], DT.bfloat16).ap()

    with tile.TileContext(nc) as tc, tc.tile_pool(name="pers", bufs=1) as pers:
        with (
            tc.tile_pool(name="route", bufs=24) as route,
            tc.tile_pool(name="routeps", bufs=2, space="PSUM") as routeps,
            tc.tile_pool(name="gateps", bufs=2, space="PSUM") as gateps,
            tc.tile_pool(name="xtp", bufs=5) as xtp,
        ):
            ident = pers.tile([P, P], DT.float32)
            make_identity(nc, ident)
            wg_sb = pers.tile([P, KD * E], DT.float32)
            nc.sync.dma_start(wg_sb[:], wg[:, :])
            bgb_sb = pers.tile([P, E], DT.float32)
            nc.sync.dma_start(bgb_sb[:], bgb[:, :])
            iot_sb = pers.tile([P, E], DT.float32)
            nc.sync.dma_start(iot_sb[:], iotae[:, :])
            toke_sb = pers.tile([16, T], DT.int16)
            nc.sync.dma_start(toke_sb[:], tokide[:, :])
            rep_sb = pers.tile([16, P], DT.float32)
            nc.sync.dma_start(rep_sb[:], repm[:, :])

            # init idxlist to the dump token id (T -> zero row of xb)
            init_t = pers.tile([P, YROWS // P], DT.int16)
            nc.vector.memset(init_t[:], T)
            nc.sync.dma_start(idxlist.rearrange("(p c) o -> p (c o)", p=P), init_t[:])

            # zero the spill rows of ybuf (rows e*STRIDE+CAP .. e*STRIDE+655)
            zspill = pers.tile([16, D], DT.bfloat16)
            nc.vector.memset(zspill[:], 0)
            for e in range(E):
                nc.sync.dma_start(
                    ybuf[e * STRIDE + CAP : e * STRIDE + STRIDE, :], zspill[:]
                )

            combT = pers.tile([16, T], DT.float32)
            nc.vector.memset(combT[:], 0.0)
            zerosE = pers.tile([16, T], DT.float32)
            nc.vector.memset(zerosE[:], 0.0)
            i1a = pers.tile([P, NT], DT.float32)
            i2a = pers.tile([P, NT], DT.float32)
            w1a = pers.tile([P, NT], DT.float32)
            w2a = pers.tile([P, NT], DT.float32)

            # ---- gate + top-2 per token tile ----
            for i in range(NT):
                xt_sb = xtp.tile([P, KD, P], DT.float32)
                nc.sync.dma_start(
                    xt_sb[:],
                    xt.rearrange("(k p) t -> p k t", p=P)[
                        :, :, i * P : (i + 1) * P
                    ],
                )
                ps_g = gateps.tile([P, E], DT.float32)
                for k in range(KD):
                    nc.tensor.matmul(
                        ps_g[:],
                        lhsT=xt_sb[:, k, :],
                        rhs=wg_sb[:, k * E : (k + 1) * E],
                        start=(k == 0),
                        stop=(k == KD - 1),
                    )
                logits = route.tile([P, E], DT.float32)
                nc.vector.tensor_add(logits[:], ps_g[:], bgb_sb[:])

                vals8 = route.tile([P, 8], DT.float32)
                idx8 = route.tile([P, 8], DT.uint32)
                nc.vector.max_with_indices(vals8[:], idx8[:], logits[:])
                nc.vector.tensor_copy(i1a[:, i : i + 1], idx8[:, 0:1])
                nc.vector.tensor_copy(i2a[:, i : i + 1], idx8[:, 1:2])

                oh1 = route.tile([P, E], DT.float32)
                nc.vector.tensor_scalar(
                    oh1[:], iot_sb[:], i1a[:, i : i + 1], None,
                    op0=mybir.AluOpType.is_equal,
                )
                oh2 = route.tile([P, E], DT.float32)
                nc.vector.tensor_scalar(
                    oh2[:], iot_sb[:], i2a[:, i : i + 1], None,
                    op0=mybir.AluOpType.is_equal,
                )
                comb = route.tile([P, E], DT.float32)
                nc.vector.tensor_add(comb[:], oh1[:], oh2[:])
                ps_t = routeps.tile([E, P], DT.float32)
                nc.tensor.transpose(ps_t[:], comb[:, :], ident[:])
                nc.vector.tensor_copy(combT[0:E, i * P : (i + 1) * P], ps_t[:])

                dm = route.tile([P, 1], DT.float32)
                nc.vector.tensor_sub(dm[:], vals8[:, 1:2], vals8[:, 0:1])
                ed = route.tile([P, 1], DT.float32)
                nc.scalar.activation(ed[:], dm[:], mybir.ActivationFunctionType.Exp)
                den = route.tile([P, 1], DT.float32)
                nc.vector.tensor_scalar_add(den[:], ed[:], 1.0)
                nc.vector.reciprocal(w1a[:, i : i + 1], den[:])
                nc.vector.tensor_mul(w2a[:, i : i + 1], ed[:], w1a[:, i : i + 1])

            # ---- cumulative per-expert counts -> slot positions ----
            incl = pers.tile([16, T], DT.float32)
            nc.vector.tensor_tensor_scan(
                incl[:], combT[:], zerosE[:], 0.0,
                op0=mybir.AluOpType.add, op1=mybir.AluOpType.add,
            )
            excl = pers.tile([16, T], DT.float32)
            nc.vector.tensor_sub(excl[:], incl[:], combT[:])

            # build per-expert token lists with one SBUF-local scatter:
            # lsdst[e, pos] = token id (0 for empty slots -> harmless compute)
            exclm = pers.tile([16, T], DT.float32)
            nc.vector.tensor_scalar_min(exclm[:], excl[:], float(STRIDE - 1))
            idnf = pers.tile([16, T], DT.float32)
            nc.vector.tensor_mul(idnf[:], exclm[:], combT[:])
            nc.vector.tensor_add(idnf[:], idnf[:], combT[:])
            nc.vector.tensor_scalar_add(idnf[:], idnf[:], -1.0)
            idn16 = pers.tile([16, T], DT.int16)
            nc.vector.tensor_copy(idn16[:], idnf[:])
            lsdst = pers.tile([16, STRIDE], DT.int16)
            nc.gpsimd.local_scatter(
                out_ap=lsdst[:],
                data_ap=toke_sb[:],
                idxs_ap=idn16[:],
                channels=16,
                num_elems=STRIDE,
                num_idxs=T,
            )
            nc.sync.dma_start(
                idxlist.rearrange("(e c) o -> e (c o)", e=E), lsdst[0:E, :]
            )

            for i in range(NT):
                ps_e = routeps.tile([P, 16], DT.float32)
                nc.tensor.transpose(
                    ps_e[:], excl[:, i * P : (i + 1) * P], ident[0:16, 0:16]
                )
                excl_tok = route.tile([P, 16], DT.float32)
                nc.vector.tensor_copy(excl_tok[:], ps_e[:])
                for slot, ifc in ((0, i1a), (1, i2a)):
                    oh = route.tile([P, E], DT.float32)
                    nc.vector.tensor_scalar(
                        oh[:], iot_sb[:], ifc[:, i : i + 1], None,
                        op0=mybir.AluOpType.is_equal,
                    )
                    tmp = route.tile([P, E], DT.float32)
                    nc.vector.tensor_mul(tmp[:], excl_tok[:, 0:E], oh[:])
                    ppos = route.tile([P, 1], DT.float32)
                    nc.vector.tensor_reduce(
                        ppos[:], tmp[:], axis=mybir.AxisListType.X,
                        op=mybir.AluOpType.add,
                    )
                    pm = route.tile([P, 1], DT.float32)
                    nc.vector.tensor_scalar_min(pm[:], ppos[:], float(CAP))
                    g = route.tile([P, 1], DT.float32)
                    nc.vector.tensor_scalar(
                        g[:], ifc[:, i : i + 1], float(STRIDE), pm[:, 0:1],
                        op0=mybir.AluOpType.mult, op1=mybir.AluOpType.add,
                    )
                    gs = route.tile([P, 1], DT.int16)
                    nc.vector.tensor_copy(gs[:], g[:])
                    nc.sync.dma_start(
                        gbuf[slot : slot + 1, i * P : (i + 1) * P], gs[:, 0:1]
                    )

        # ---- replicated wrapped idx panel (one per kernel) ----
        # idxrep[q, e*41+c] = idxlist[e*656 + c*16 + (q%16)]; partition-group
        # replication done on PE via the 0/1 matrix rep_sb.
        with tc.tile_pool(name="idxrep_ps", bufs=1, space="PSUM") as irps:
            idxw = pers.tile([16, E * 41], DT.int16)
            with nc.allow_non_contiguous_dma(reason="wrapped idx load, 10KB once"):
                nc.sync.dma_start(
                    idxw[:, :],
                    idxlist.rearrange("(e c p) o -> p (e c o)", p=16, c=41),
                )
            idxwf = pers.tile([16, E * 41], DT.float32)
            nc.vector.tensor_copy(idxwf[:], idxw[:])
            ip1 = irps.tile([P, 164], DT.float32)
            nc.tensor.matmul(ip1[:], lhsT=rep_sb[:], rhs=idxwf[:, 0:164],
                             start=True, stop=True)
            ip2 = irps.tile([P, 164], DT.float32)
            nc.tensor.matmul(ip2[:], lhsT=rep_sb[:], rhs=idxwf[:, 164:328],
                             start=True, stop=True)
            idxrep = pers.tile([P, E * 41], DT.int16)
            nc.vector.tensor_copy(idxrep[:, 0:164], ip1[:])
            nc.vector.tensor_copy(idxrep[:, 164:328], ip2[:])

        # ---- expert loop ----
        with (
            tc.tile_pool(name="w1p", bufs=2) as w1p,
            tc.tile_pool(name="w2p", bufs=2) as w2p,
            tc.tile_pool(name="bp", bufs=2) as bp,
            tc.tile_pool(name="xg", bufs=2) as xg,
            tc.tile_pool(name="hp", bufs=1) as hp,
            tc.tile_pool(name="yp", bufs=2) as yp,
            tc.tile_pool(name="ytk", bufs=2) as ytk,
            tc.tile_pool(name="mm1ps", bufs=3, space="PSUM") as mm1ps,
            tc.tile_pool(name="mm2ps", bufs=3, space="PSUM") as mm2ps,
        ):
            for e in range(E):
                xgT = xg.tile([P, KD, CAP], DT.bfloat16)
                nc.gpsimd.dma_gather(
                    out_ap=xgT[:],
                    in_ap=xb[:, :],
                    idxs_ap=idxrep[:, e * 41 : e * 41 + CAP // 16],
                    num_idxs=CAP,
                    num_idxs_reg=CAP,
                    elem_size=D,
                    transpose=True,
                )

                w1t = []
                for t in range(2):
                    wt = w1p.tile([P, 4 * H], DT.bfloat16, tag="w1")
                    nc.scalar.dma_start(wt[:], w1l[e, :, t * 4 * H : (t + 1) * 4 * H])
                    w1t.append(wt)
                w2t = []
                for t in range(2):
                    wt = w2p.tile([P, 8 * D], DT.bfloat16, tag="w2")
                    nc.scalar.dma_start(wt[:], w2l[e, :, t * 8 * D : (t + 1) * 8 * D])
                    w2t.append(wt)
                w1k = [w1t[k // 4][:, (k % 4) * H : (k % 4) * H + H] for k in range(KD)]
                w2k = [w2t[k // 8][:, (k % 8) * D : (k % 8) * D + D] for k in range(KH)]
                b1t = bp.tile([P, MH], DT.float32, tag="b1")
                nc.scalar.dma_start(b1t[:], b1l[e, :, :])
                b2t = bp.tile([P, MD], DT.float32, tag="b2")
                nc.scalar.dma_start(b2t[:], b2l[e, :, :])

                hT = [
                    hp.tile([P, CAP], DT.bfloat16, tag=f"hT{m}", name=f"hT{m}")
                    for m in range(MH)
                ]
                for m in range(MH):
                    for n0, nsz in NCH:
                        ps1 = mm1ps.tile([P, 512], DT.float32)
                        for k in range(KD):
                            nc.tensor.matmul(
                                ps1[:, :nsz],
                                lhsT=w1k[k][:, m * P : (m + 1) * P],
                                rhs=xgT[:, k, n0 : n0 + nsz],
                                start=(k == 0),
                                stop=(k == KD - 1),
                            )
                        nc.scalar.activation(
                            hT[m][:, n0 : n0 + nsz],
                            ps1[:, :nsz],
                            mybir.ActivationFunctionType.Relu,
                            bias=b1t[:, m : m + 1],
                        )

                yT = [
                    yp.tile([P, CAP], DT.bfloat16, tag=f"yT{md}", name=f"yT{md}")
                    for md in range(MD)
                ]
                for md in range(MD):
                    for n0, nsz in NCH:
                        ps2 = mm2ps.tile([P, 512], DT.float32)
                        for k in range(KH):
                            nc.tensor.matmul(
                                ps2[:, :nsz],
                                lhsT=w2k[k][:, md * P : (md + 1) * P],
                                rhs=hT[k][:, n0 : n0 + nsz],
                                start=(k == 0),
                                stop=(k == KH - 1),
                            )
                        nc.scalar.activation(
                            yT[md][:, n0 : n0 + nsz],
                            ps2[:, :nsz],
                            mybir.ActivationFunctionType.Identity,
                            bias=b2t[:, md : md + 1],
                        )

                ytok = ytk.tile([P, CAP // P, D], DT.bfloat16)
                for md in range(MD):
                    nc.sync.dma_start_transpose(
                        ytok[:, :, md * P : (md + 1) * P],
                        yT[md][:, :],
                    )
                nc.sync.dma_start(
                    ybuf[e * STRIDE : e * STRIDE + CAP, :].rearrange(
                        "(c p) d -> p c d", p=P
                    ),
                    ytok[:],
                )

        # ---- final combine ----
        with (
            tc.tile_pool(name="fin", bufs=4) as fin,
            tc.tile_pool(name="fing", bufs=1) as fing,
            tc.tile_pool(name="finps", bufs=2, space="PSUM") as finps,
        ):
            gth = []
            for slot in range(2):
                gw = fin.tile([16, NT * 8], DT.int16, tag="gw")
                with nc.allow_non_contiguous_dma(reason="wrapped idx load, 4KB once"):
                    nc.sync.dma_start(
                        gw[:, :],
                        gbuf[slot, :].rearrange("(c p) -> p c", p=16),
                    )
                gwf = fin.tile([16, NT * 8], DT.float32, tag="gwf")
                nc.vector.tensor_copy(gwf[:], gw[:])
                gps = finps.tile([P, NT * 8], DT.float32)
                nc.tensor.matmul(gps[:], lhsT=rep_sb[:], rhs=gwf[:],
                                 start=True, stop=True)
                grep = fin.tile([P, NT * 8], DT.int16, tag="grep")
                nc.vector.tensor_copy(grep[:], gps[:])
                # dma_gather dies above 512 idxs (non-transpose mode), so
                # fetch in 512-token chunks
                gts = []
                for ch in range(NT // 4):
                    gt = fing.tile([P, 4, D], DT.bfloat16, tag=f"gth{slot}_{ch}",
                                   name=f"gth{slot}_{ch}")
                    nc.gpsimd.dma_gather(
                        out_ap=gt[:],
                        in_ap=ybuf[:, :],
                        idxs_ap=grep[:, ch * 32 : (ch + 1) * 32],
                        num_idxs=512,
                        num_idxs_reg=512,
                        elem_size=D,
                    )
                    gts.append(gt)
                gth.append(gts)
            for i in range(NT):
                acc1 = fin.tile([P, D], DT.float32, tag="acc1")
                nc.vector.tensor_scalar(
                    acc1[:], gth[0][i // 4][:, i % 4, :], w1a[:, i : i + 1], None,
                    op0=mybir.AluOpType.mult,
                )
                acc2 = fin.tile([P, D], DT.float32, tag="acc2")
                nc.vector.tensor_scalar(
                    acc2[:], gth[1][i // 4][:, i % 4, :], w2a[:, i : i + 1], None,
                    op0=mybir.AluOpType.mult,
                )
                res = fin.tile([P, D], DT.float32, tag="res")
                nc.vector.tensor_add(res[:], acc1[:], acc2[:])
                nc.sync.dma_start(out[i * P : (i + 1) * P, :], res[:])

    nc.compile()
    return nc


def prep_inputs(x, Wg, bg, W1, b1, W2, b2):
    """Build the 8 per-core input maps from full problem inputs (numpy f32)."""
    bf16 = ml_dtypes.bfloat16
    wg_l = np.ascontiguousarray(
        Wg.reshape(KD, P, E).transpose(1, 0, 2).reshape(P, KD * E)
    )
    bgb_np = np.tile(bg[None, :], (P, 1)).astype(np.float32)
    iot_np = np.tile(np.arange(E, dtype=np.float32)[None, :], (P, 1))
    toke_np = np.tile(np.arange(T, dtype=np.int16)[None, :], (16, 1))
    rep_np = (np.arange(P)[None, :] % 16 == np.arange(16)[:, None]).astype(np.float32)
    w1l_np = np.ascontiguousarray(
        W1.reshape(E, KD, P, H).transpose(0, 2, 1, 3).reshape(E, P, KD * H)
    ).astype(bf16)
    w2l_np = np.ascontiguousarray(
        W2.reshape(E, KH, P, D).transpose(0, 2, 1, 3).reshape(E, P, KH * D)
    ).astype(bf16)
    b1l_np = np.ascontiguousarray(
        b1.reshape(E, MH, P).transpose(0, 2, 1)
    ).astype(np.float32)
    b2l_np = np.ascontiguousarray(
        b2.reshape(E, MD, P).transpose(0, 2, 1)
    ).astype(np.float32)

    in_maps = []
    for c in range(B):
        xc = np.asarray(x[c], dtype=np.float32)  # [T, D]
        xt_np = np.ascontiguousarray(xc.T)
        xb_np = np.zeros((XROWS, D), dtype=bf16)
        xb_np[:T] = xc.astype(bf16)
        in_maps.append(
            {
                "xt": xt_np,
                "xb": xb_np,
                "wg": wg_l,
                "bgb": bgb_np,
                "iotae": iot_np,
                "tokide": toke_np,
                "repm": rep_np,
                "w1l": w1l_np,
                "w2l": w2l_np,
                "b1l": b1l_np,
                "b2l": b2l_np,
            }
        )
    return in_maps


_nc_cache = None


def kernel(**inputs):
    global _nc_cache
    from concourse.bass_utils import run_bass_kernel_spmd

    if _nc_cache is None:
        _nc_cache = build_program()
    nc = _nc_cache
    in_maps = prep_inputs(
        np.asarray(inputs["x"], dtype=np.float32),
        np.asarray(inputs["Wg"], dtype=np.float32),
        np.asarray(inputs["bg"], dtype=np.float32),
        np.asarray(inputs["W1"], dtype=np.float32),
        np.asarray(inputs["b1"], dtype=np.float32),
        np.asarray(inputs["W2"], dtype=np.float32),
        np.asarray(inputs["b2"], dtype=np.float32),
    )
    res = run_bass_kernel_spmd(nc, in_maps, core_ids=list(range(B)))
    out = np.stack([res.results[c]["out"] for c in range(B)], axis=0)
    return out.astype(np.float32)


# revision 13
# speedup vs baseline: 2.3771x; 1.0598x over previous
"""Trainium2 Bass kernel for an 8-expert top-2 MoE layer.

Problem (hardcoded): x[8,2048,1024] f32, gate Wg[1024,8]+bg, experts
W1[8,1024,2048]+b1, W2[8,2048,1024]+b2, top-2 routing with renormalized
gate weights, out[8,2048,1024] f32.

Strategy: data-parallel over tokens. Each of the 8 NeuronCores processes one
batch row (2048 tokens) with all experts resident:
  1. gate logits via PE (fp32), top-2 + weights via DVE max8,
  2. build per-expert token lists on-device (one-hot transpose -> free-axis
     cumsum -> positions -> indirect scatter of token ids),
  3. per expert: dma_gather(transpose) dispatches routed tokens into a
     [D,tok] bf16 activation panel; two bf16 matmuls (weights stationary as
     lhsT) with fused bias+ReLU eviction; xbar DMA-transpose back to
     token-major; linear store into a [expert-slot, D] bf16 workspace,
  4. final combine: per token dma_gather of its two expert rows, scale by
     gate weights in fp32, store.
The capacity per (core, expert) is CAP=640 slots (mean load 512); overflow
beyond CAP is clamped into an unprocessed spill slot (probability ~0 for
gaussian inputs).
"""

import sys

for _p in ("/opt/trn_rl_repo",):
    if _p not in sys.path:
        sys.path.append(_p)

import numpy as np
import ml_dtypes

import concourse.bass as bass
import concourse.bacc as bacc
import concourse.tile as tile
import concourse.mybir as mybir
from concourse.masks import make_identity

P = 128
B, S, D = 8, 2048, 1024
E, H, TOPK = 8, 2048, 2
T = S  # tokens per core (one batch row per core)
NT = T // P  # 16 token tiles
KD = D // P  # 8 contraction tiles for D
KH = H // P  # 16 contraction tiles for H
MH = H // P  # 16 output tiles for H
MD = D // P  # 8 output tiles for D
CAP = 640  # processed slots per (core, expert)
STRIDE = 656  # idxlist/ybuf row stride per expert (CAP + spill)
XROWS = T + 16  # xb pad rows; row T is the all-zero dump row
YROWS = E * STRIDE
NCH = ((0, 512), (512, 128))  # token chunks of CAP for PSUM banks
DT = mybir.dt


def build_program():
    nc = bacc.Bacc("TRN2", target_bir_lowering=False, debug=False, num_devices=8)

    xt = nc.dram_tensor("xt", [D, T], DT.float32, kind="ExternalInput").ap()
    xb = nc.dram_tensor("xb", [XROWS, D], DT.bfloat16, kind="ExternalInput").ap()
    wg = nc.dram_tensor("wg", [P, KD * E], DT.float32, kind="ExternalInput").ap()
    bgb = nc.dram_tensor("bgb", [P, E], DT.float32, kind="ExternalInput").ap()
    iotae = nc.dram_tensor("iotae", [P, E], DT.float32, kind="ExternalInput").ap()
    tokide = nc.dram_tensor("tokide", [16, T], DT.int16, kind="ExternalInput").ap()
    repm = nc.dram_tensor("repm", [16, P], DT.float32, kind="ExternalInput").ap()
    w1l = nc.dram_tensor("w1l", [E, P, KD * H], DT.bfloat16, kind="ExternalInput").ap()
    w2l = nc.dram_tensor("w2l", [E, P, KH * D], DT.bfloat16, kind="ExternalInput").ap()
    b1l = nc.dram_tensor("b1l", [E, P, MH], DT.float32, kind="ExternalInput").ap()
    b2l = nc.dram_tensor("b2l", [E, P, MD], DT.float32, kind="ExternalInput").ap()
    out = nc.dram_tensor("out", [T, D], DT.float32, kind="ExternalOutput").ap()

    idxlist = nc.dram_tensor("idxlist", [YROWS, 1], DT.int16).ap()
    gbuf = nc.dram_tensor("gbuf", [2, P, NT], DT.int16).ap()
    ybuf = nc.dram_tensor("ybuf", [YROWS, D], DT.bfloat16).ap()

    with tile.TileContext(nc) as tc, tc.tile_pool(name="pers", bufs=1) as pers:
        with (
            tc.tile_pool(name="route", bufs=24) as route,
            tc.tile_pool(name="routeps", bufs=2, space="PSUM") as routeps,
            tc.tile_pool(name="gateps", bufs=2, space="PSUM") as gateps,
            tc.tile_pool(name="xtp", bufs=5) as xtp,
        ):
            ident = pers.tile([P, P], DT.float32)
            make_identity(nc, ident)
            wg_sb = pers.tile([P, KD * E], DT.float32)
            nc.sync.dma_start(wg_sb[:], wg[:, :])
            bgb_sb = pers.tile([P, E], DT.float32)
            nc.sync.dma_start(bgb_sb[:], bgb[:, :])
            iot_sb = pers.tile([P, E], DT.float32)
            nc.sync.dma_start(iot_sb[:], iotae[:, :])
            toke_sb = pers.tile([16, T], DT.int16)
            nc.sync.dma_start(toke_sb[:], tokide[:, :])
            rep_sb = pers.tile([16, P], DT.float32)
            nc.sync.dma_start(rep_sb[:], repm[:, :])

            # init idxlist to the dump token id (T -> zero row of xb)
            init_t = pers.tile([P, YROWS // P], DT.int16)
            nc.vector.memset(init_t[:], T)
            nc.sync.dma_start(idxlist.rearrange("(p c) o -> p (c o)", p=P), init_t[:])

            # zero the spill rows of ybuf (rows e*STRIDE+CAP .. e*STRIDE+655)
            zspill = pers.tile([16, D], DT.bfloat16)
            nc.vector.memset(zspill[:], 0)
            for e in range(E):
                nc.sync.dma_start(
                    ybuf[e * STRIDE + CAP : e * STRIDE + STRIDE, :], zspill[:]
                )

            combT = pers.tile([16, T], DT.float32)
            nc.vector.memset(combT[:], 0.0)
            zerosE = pers.tile([16, T], DT.float32)
            nc.vector.memset(zerosE[:], 0.0)
            i1a = pers.tile([P, NT], DT.float32)
            i2a = pers.tile([P, NT], DT.float32)
            w1a = pers.tile([P, NT], DT.float32)
            w2a = pers.tile([P, NT], DT.float32)
            gsa0 = pers.tile([P, NT], DT.int16)
            gsa1 = pers.tile([P, NT], DT.int16)

            # ---- gate + top-2 per token tile ----
            for i in range(NT):
                xt_sb = xtp.tile([P, KD, P], DT.float32)
                nc.sync.dma_start(
                    xt_sb[:],
                    xt.rearrange("(k p) t -> p k t", p=P)[
                        :, :, i * P : (i + 1) * P
                    ],
                )
                ps_g = gateps.tile([P, E], DT.float32)
                for k in range(KD):
                    nc.tensor.matmul(
                        ps_g[:],
                        lhsT=xt_sb[:, k, :],
                        rhs=wg_sb[:, k * E : (k + 1) * E],
                        start=(k == 0),
                        stop=(k == KD - 1),
                    )
                logits = route.tile([P, E], DT.float32)
                nc.vector.tensor_add(logits[:], ps_g[:], bgb_sb[:])

                vals8 = route.tile([P, 8], DT.float32)
                idx8 = route.tile([P, 8], DT.uint32)
                nc.vector.max_with_indices(vals8[:], idx8[:], logits[:])
                nc.vector.tensor_copy(i1a[:, i : i + 1], idx8[:, 0:1])
                nc.vector.tensor_copy(i2a[:, i : i + 1], idx8[:, 1:2])

                oh1 = route.tile([P, E], DT.float32)
                nc.vector.tensor_scalar(
                    oh1[:], iot_sb[:], i1a[:, i : i + 1], None,
                    op0=mybir.AluOpType.is_equal,
                )
                oh2 = route.tile([P, E], DT.float32)
                nc.vector.tensor_scalar(
                    oh2[:], iot_sb[:], i2a[:, i : i + 1], None,
                    op0=mybir.AluOpType.is_equal,
                )
                comb = route.tile([P, E], DT.float32)
                nc.vector.tensor_add(comb[:], oh1[:], oh2[:])
                ps_t = routeps.tile([E, P], DT.float32)
                nc.tensor.transpose(ps_t[:], comb[:, :], ident[:])
                nc.vector.tensor_copy(combT[0:E, i * P : (i + 1) * P], ps_t[:])

                dm = route.tile([P, 1], DT.float32)
                nc.vector.tensor_sub(dm[:], vals8[:, 1:2], vals8[:, 0:1])
                ed = route.tile([P, 1], DT.float32)
                nc.scalar.activation(ed[:], dm[:], mybir.ActivationFunctionType.Exp)
                den = route.tile([P, 1], DT.float32)
                nc.vector.tensor_scalar_add(den[:], ed[:], 1.0)
                nc.vector.reciprocal(w1a[:, i : i + 1], den[:])
                nc.vector.tensor_mul(w2a[:, i : i + 1], ed[:], w1a[:, i : i + 1])

            # ---- cumulative per-expert counts -> slot positions ----
            incl = pers.tile([16, T], DT.float32)
            nc.vector.tensor_tensor_scan(
                incl[:], combT[:], zerosE[:], 0.0,
                op0=mybir.AluOpType.add, op1=mybir.AluOpType.add,
            )
            excl = pers.tile([16, T], DT.float32)
            nc.vector.tensor_sub(excl[:], incl[:], combT[:])

            # build per-expert token lists with one SBUF-local scatter:
            # lsdst[e, pos] = token id (0 for empty slots -> harmless compute)
            exclm = pers.tile([16, T], DT.float32)
            nc.vector.tensor_scalar_min(exclm[:], excl[:], float(STRIDE - 1))
            idnf = pers.tile([16, T], DT.float32)
            nc.vector.tensor_mul(idnf[:], exclm[:], combT[:])
            nc.vector.tensor_add(idnf[:], idnf[:], combT[:])
            nc.vector.tensor_scalar_add(idnf[:], idnf[:], -1.0)
            idn16 = pers.tile([16, T], DT.int16)
            nc.vector.tensor_copy(idn16[:], idnf[:])
            lsdst = pers.tile([16, STRIDE], DT.int16)
            nc.gpsimd.local_scatter(
                out_ap=lsdst[:],
                data_ap=toke_sb[:],
                idxs_ap=idn16[:],
                channels=16,
                num_elems=STRIDE,
                num_idxs=T,
            )
            nc.sync.dma_start(
                idxlist.rearrange("(e c) o -> e (c o)", e=E), lsdst[0:E, :]
            )

            for i in range(NT):
                ps_e = routeps.tile([P, 16], DT.float32)
                nc.tensor.transpose(
                    ps_e[:], excl[:, i * P : (i + 1) * P], ident[0:16, 0:16]
                )
                excl_tok = route.tile([P, 16], DT.float32)
                nc.vector.tensor_copy(excl_tok[:], ps_e[:])
                for slot, ifc in ((0, i1a), (1, i2a)):
                    oh = route.tile([P, E], DT.float32)
                    nc.vector.tensor_scalar(
                        oh[:], iot_sb[:], ifc[:, i : i + 1], None,
                        op0=mybir.AluOpType.is_equal,
                    )
                    tmp = route.tile([P, E], DT.float32)
                    nc.vector.tensor_mul(tmp[:], excl_tok[:, 0:E], oh[:])
                    ppos = route.tile([P, 1], DT.float32)
                    nc.vector.tensor_reduce(
                        ppos[:], tmp[:], axis=mybir.AxisListType.X,
                        op=mybir.AluOpType.add,
                    )
                    pm = route.tile([P, 1], DT.float32)
                    nc.vector.tensor_scalar_min(pm[:], ppos[:], float(CAP))
                    g = route.tile([P, 1], DT.float32)
                    nc.vector.tensor_scalar(
                        g[:], ifc[:, i : i + 1], float(STRIDE), pm[:, 0:1],
                        op0=mybir.AluOpType.mult, op1=mybir.AluOpType.add,
                    )
                    gsa = gsa0 if slot == 0 else gsa1
                    nc.vector.tensor_copy(gsa[:, i : i + 1], g[:])

            nc.sync.dma_start(gbuf[0, :, :], gsa0[:])
            nc.sync.dma_start(gbuf[1, :, :], gsa1[:])

        # ---- replicated wrapped idx panel (one per kernel) ----
        # idxrep[q, e*41+c] = idxlist[e*656 + c*16 + (q%16)]; partition-group
        # replication done on PE via the 0/1 matrix rep_sb.
        with tc.tile_pool(name="idxrep_ps", bufs=1, space="PSUM") as irps:
            idxw = pers.tile([16, E * 41], DT.int16)
            with nc.allow_non_contiguous_dma(reason="wrapped idx load, 10KB once"):
                nc.sync.dma_start(
                    idxw[:, :],
                    idxlist.rearrange("(e c p) o -> p (e c o)", p=16, c=41),
                )
            idxwf = pers.tile([16, E * 41], DT.float32)
            nc.vector.tensor_copy(idxwf[:], idxw[:])
            ip1 = irps.tile([P, 164], DT.float32)
            nc.tensor.matmul(ip1[:], lhsT=rep_sb[:], rhs=idxwf[:, 0:164],
                             start=True, stop=True)
            ip2 = irps.tile([P, 164], DT.float32)
            nc.tensor.matmul(ip2[:], lhsT=rep_sb[:], rhs=idxwf[:, 164:328],
                             start=True, stop=True)
            idxrep = pers.tile([P, E * 41], DT.int16)
            nc.vector.tensor_copy(idxrep[:, 0:164], ip1[:])
            nc.vector.tensor_copy(idxrep[:, 164:328], ip2[:])

        # ---- expert loop ----
        with (
            tc.tile_pool(name="w1p", bufs=2) as w1p,
            tc.tile_pool(name="w2p", bufs=2) as w2p,
            tc.tile_pool(name="bp", bufs=2) as bp,
            tc.tile_pool(name="xg", bufs=2) as xg,
            tc.tile_pool(name="hp", bufs=1) as hp,
            tc.tile_pool(name="yp", bufs=2) as yp,
            tc.tile_pool(name="ytk", bufs=2) as ytk,
            tc.tile_pool(name="mm1ps", bufs=3, space="PSUM") as mm1ps,
            tc.tile_pool(name="mm2ps", bufs=3, space="PSUM") as mm2ps,
        ):
            for e in range(E):
                xgT = xg.tile([P, KD, CAP], DT.bfloat16)
                nc.gpsimd.dma_gather(
                    out_ap=xgT[:],
                    in_ap=xb[:, :],
                    idxs_ap=idxrep[:, e * 41 : e * 41 + CAP // 16],
                    num_idxs=CAP,
                    num_idxs_reg=CAP,
                    elem_size=D,
                    transpose=True,
                )

                w1t = []
                for t in range(2):
                    wt = w1p.tile([P, 4 * H], DT.bfloat16, tag="w1")
                    nc.scalar.dma_start(wt[:], w1l[e, :, t * 4 * H : (t + 1) * 4 * H])
                    w1t.append(wt)
                w2t = []
                for t in range(2):
                    wt = w2p.tile([P, 8 * D], DT.bfloat16, tag="w2")
                    nc.scalar.dma_start(wt[:], w2l[e, :, t * 8 * D : (t + 1) * 8 * D])
                    w2t.append(wt)
                w1k = [w1t[k // 4][:, (k % 4) * H : (k % 4) * H + H] for k in range(KD)]
                w2k = [w2t[k // 8][:, (k % 8) * D : (k % 8) * D + D] for k in range(KH)]
                b1t = bp.tile([P, MH], DT.float32, tag="b1")
                nc.scalar.dma_start(b1t[:], b1l[e, :, :])
                b2t = bp.tile([P, MD], DT.float32, tag="b2")
                nc.scalar.dma_start(b2t[:], b2l[e, :, :])

                hT = [
                    hp.tile([P, CAP], DT.bfloat16, tag=f"hT{m}", name=f"hT{m}")
                    for m in range(MH)
                ]
                for m in range(MH):
                    for n0, nsz in NCH:
                        ps1 = mm1ps.tile([P, 512], DT.float32)
                        for k in range(KD):
                            nc.tensor.matmul(
                                ps1[:, :nsz],
                                lhsT=w1k[k][:, m * P : (m + 1) * P],
                                rhs=xgT[:, k, n0 : n0 + nsz],
                                start=(k == 0),
                                stop=(k == KD - 1),
                            )
                        nc.scalar.activation(
                            hT[m][:, n0 : n0 + nsz],
                            ps1[:, :nsz],
                            mybir.ActivationFunctionType.Relu,
                            bias=b1t[:, m : m + 1],
                        )

                yT = [
                    yp.tile([P, CAP], DT.bfloat16, tag=f"yT{md}", name=f"yT{md}")
                    for md in range(MD)
                ]
                for md in range(MD):
                    for n0, nsz in NCH:
                        ps2 = mm2ps.tile([P, 512], DT.float32)
                        for k in range(KH):
                            nc.tensor.matmul(
                                ps2[:, :nsz],
                                lhsT=w2k[k][:, md * P : (md + 1) * P],
                                rhs=hT[k][:, n0 : n0 + nsz],
                                start=(k == 0),
                                stop=(k == KH - 1),
                            )
                        nc.scalar.activation(
                            yT[md][:, n0 : n0 + nsz],
                            ps2[:, :nsz],
                            mybir.ActivationFunctionType.Identity,
                            bias=b2t[:, md : md + 1],
                        )

                ytok = ytk.tile([P, CAP // P, D], DT.bfloat16)
                for md in range(MD):
                    nc.sync.dma_start_transpose(
                        ytok[:, :, md * P : (md + 1) * P],
                        yT[md][:, :],
                    )
                nc.sync.dma_start(
                    ybuf[e * STRIDE : e * STRIDE + CAP, :].rearrange(
                        "(c p) d -> p c d", p=P
                    ),
                    ytok[:],
                )

        # ---- final combine ----
        with (
            tc.tile_pool(name="fin", bufs=4) as fin,
            tc.tile_pool(name="fing", bufs=1) as fing,
            tc.tile_pool(name="finps", bufs=2, space="PSUM") as finps,
        ):
            gth = []
            for slot in range(2):
                gw = fin.tile([16, NT * 8], DT.int16, tag="gw")
                with nc.allow_non_contiguous_dma(reason="wrapped idx load, 4KB once"):
                    nc.sync.dma_start(
                        gw[:, :].rearrange("r (i b) -> r i b", b=8),
                        gbuf[slot, :, :].rearrange("(b r) i -> r i b", b=8),
                    )
                gwf = fin.tile([16, NT * 8], DT.float32, tag="gwf")
                nc.vector.tensor_copy(gwf[:], gw[:])
                gps = finps.tile([P, NT * 8], DT.float32)
                nc.tensor.matmul(gps[:], lhsT=rep_sb[:], rhs=gwf[:],
                                 start=True, stop=True)
                grep = fin.tile([P, NT * 8], DT.int16, tag="grep")
                nc.vector.tensor_copy(grep[:], gps[:])
                # dma_gather dies above 512 idxs (non-transpose mode), so
                # fetch in 512-token chunks
                gts = []
                for ch in range(NT // 4):
                    gt = fing.tile([P, 4, D], DT.bfloat16, tag=f"gth{slot}_{ch}",
                                   name=f"gth{slot}_{ch}")
                    nc.gpsimd.dma_gather(
                        out_ap=gt[:],
                        in_ap=ybuf[:, :],
                        idxs_ap=grep[:, ch * 32 : (ch + 1) * 32],
                        num_idxs=512,
                        num_idxs_reg=512,
                        elem_size=D,
                    )
                    gts.append(gt)
                gth.append(gts)
            for i in range(NT):
                acc1 = fin.tile([P, D], DT.float32, tag="acc1")
                nc.vector.tensor_scalar(
                    acc1[:], gth[0][i // 4][:, i % 4, :], w1a[:, i : i + 1], None,
                    op0=mybir.AluOpType.mult,
                )
                acc2 = fin.tile([P, D], DT.float32, tag="acc2")
                nc.scalar.mul(acc2[:], gth[1][i // 4][:, i % 4, :], w2a[:, i : i + 1])
                res = fin.tile([P, D], DT.float32, tag="res")
                nc.vector.tensor_add(res[:], acc1[:], acc2[:])
                nc.sync.dma_start(out[i * P : (i + 1) * P, :], res[:])

    nc.compile()
    return nc


def prep_inputs(x, Wg, bg, W1, b1, W2, b2):
    """Build the 8 per-core input maps from full problem inputs (numpy f32)."""
    bf16 = ml_dtypes.bfloat16
    wg_l = np.ascontiguousarray(
        Wg.reshape(KD, P, E).transpose(1, 0, 2).reshape(P, KD * E)
    )
    bgb_np = np.tile(bg[None, :], (P, 1)).astype(np.float32)
    iot_np = np.tile(np.arange(E, dtype=np.float32)[None, :], (P, 1))
    toke_np = np.tile(np.arange(T, dtype=np.int16)[None, :], (16, 1))
    rep_np = (np.arange(P)[None, :] % 16 == np.arange(16)[:, None]).astype(np.float32)
    w1l_np = np.ascontiguousarray(
        W1.reshape(E, KD, P, H).transpose(0, 2, 1, 3).reshape(E, P, KD * H)
    ).astype(bf16)
    w2l_np = np.ascontiguousarray(
        W2.reshape(E, KH, P, D).transpose(0, 2, 1, 3).reshape(E, P, KH * D)
    ).astype(bf16)
    b1l_np = np.ascontiguousarray(
        b1.reshape(E, MH, P).transpose(0, 2, 1)
    ).astype(np.float32)
    b2l_np = np.ascontiguousarray(
        b2.reshape(E, MD, P).transpose(0, 2, 1)
    ).astype(np.float32)

    in_maps = []
    for c in range(B):
        xc = np.asarray(x[c], dtype=np.float32)  # [T, D]
        xt_np = np.ascontiguousarray(xc.T)
        xb_np = np.zeros((XROWS, D), dtype=bf16)
        xb_np[:T] = xc.astype(bf16)
        in_maps.append(
            {
                "xt": xt_np,
                "xb": xb_np,
                "wg": wg_l,
                "bgb": bgb_np,
                "iotae": iot_np,
                "tokide": toke_np,
                "repm": rep_np,
                "w1l": w1l_np,
                "w2l": w2l_np,
                "b1l": b1l_np,
                "b2l": b2l_np,
            }
        )
    return in_maps


_nc_cache = None


def kernel(**inputs):
    global _nc_cache
    from concourse.bass_utils import run_bass_kernel_spmd

    if _nc_cache is None:
        _nc_cache = build_program()
    nc = _nc_cache
    in_maps = prep_inputs(
        np.asarray(inputs["x"], dtype=np.float32),
        np.asarray(inputs["Wg"], dtype=np.float32),
        np.asarray(inputs["bg"], dtype=np.float32),
        np.asarray(inputs["W1"], dtype=np.float32),
        np.asarray(inputs["b1"], dtype=np.float32),
        np.asarray(inputs["W2"], dtype=np.float32),
        np.asarray(inputs["b2"], dtype=np.float32),
    )
    res = run_bass_kernel_spmd(nc, in_maps, core_ids=list(range(B)))
    out = np.stack([res.results[c]["out"] for c in range(B)], axis=0)
    return out.astype(np.float32)
